# revision 1
# baseline (speedup 1.0000x reference)
"""EdgeCNN (DGCNN) Bass/Tile kernel for TRN2 — one batch element per core.

Per edge-conv layer (N=1024 points, K=20 neighbors):
  1. PE: packed-key matmul  pd[n,j] = 2<xn,xj> - S[j] - S[n]   (PSUM, fp32)
  2. DVE: AND-mask low 10 mantissa bits, OR in column index j  -> packed keys
  3. DVE: 3x max8 + 2x match_replace -> top-20 packed keys; extract j
  4. idx -> DRAM -> read back in dma_gather wrapped layout (partition = n%16)
  5. SWDGE dma_gather rows of a = x @ (g~ Wn)^T; DVE strided reduce_max over k
  6. PE: transpose(m) + c-matmul (c = x @ (g~(Wc-Wn))^T + b) accumulated in PSUM
  7. ACT: leaky-relu (Prelu alpha=0.2) PSUM -> next layer xT
Head: conv5 via K-chunk accumulation, global max-pool, 3 FC layers on PE.
"""

import contextlib

import numpy as np

import concourse.bass as bass
import concourse.bacc as bacc
import concourse.mybir as mybir
from concourse.tile import TileContext
from concourse.masks import make_identity

F32 = mybir.dt.float32
U32 = mybir.dt.uint32
I16 = mybir.dt.int16
F16 = mybir.dt.float16
AF = mybir.ActivationFunctionType
ALU = mybir.AluOpType
AX = mybir.AxisListType

N = 1024
KNN = 20
NT = 8
NEG_SLOPE = 0.2
BNI = np.float32(1.0 / np.sqrt(1.0 + 1e-5))
LAYERS = [(3, 64), (64, 64), (64, 128), (128, 256)]
NEG_BIG = -3.0e38


def host_prep(inp):
    """Fold BN scale/bias into weights; transpose for device layout."""
    d = {}
    for li, (C, O) in enumerate(LAYERS, start=1):
        W = inp[f'W{li}'].astype(np.float32)
        g = inp[f'g{li}'].astype(np.float32)
        b = inp[f'b{li}'].astype(np.float32)
        gt = g * BNI
        Wn = W[:, :C]
        Wc = W[:, C:]
        d[f'wnt{li}'] = np.ascontiguousarray((gt[:, None] * Wn).T)          # (C, O)
        d[f'wdt{li}'] = np.ascontiguousarray((gt[:, None] * (Wc - Wn)).T)   # (C, O)
        d[f'bs{li}'] = b.reshape(1, O).copy()
    g5 = inp['g5'].astype(np.float32) * BNI
    d['w5t'] = np.ascontiguousarray((g5[:, None] * inp['W5']).T)            # (512, 512)
    d['b5'] = inp['b5'].reshape(1, 512).astype(np.float32).copy()
    g1 = inp['bng1'].astype(np.float32) * BNI
    d['wfc1'] = np.ascontiguousarray((g1[:, None] * inp['fc1_w']).T)        # (512, 256)
    bf1 = g1 * inp['fc1_b'].astype(np.float32) + inp['bnb1'].astype(np.float32)
    d['bfc1'] = np.ascontiguousarray(bf1.reshape(2, 128).T)                 # (128, 2)
    g2 = inp['bng2'].astype(np.float32) * BNI
    d['wfc2'] = np.ascontiguousarray((g2[:, None] * inp['fc2_w']).T)        # (256, 128)
    bf2 = g2 * inp['fc2_b'].astype(np.float32) + inp['bnb2'].astype(np.float32)
    d['bfc2'] = np.ascontiguousarray(bf2.reshape(128, 1))                   # (128, 1)
    d['wfc3'] = np.ascontiguousarray(inp['fc3_w'].T)                        # (128, 40)
    d['bfc3'] = inp['fc3_b'].reshape(1, 40).astype(np.float32).copy()
    return d


def build_nc(stage='full'):
    nc = bacc.Bacc("TRN2", target_bir_lowering=False, debug=False, num_devices=8)
    with TileContext(nc) as tc:
        _trace(nc, tc, stage)
    nc.compile()
    return nc


def _trace(nc, tc, stage='full'):
    with contextlib.ExitStack() as ctx:
        dram = ctx.enter_context(tc.tile_pool(name="dram", bufs=1, space="DRAM"))
        consts = ctx.enter_context(tc.tile_pool(name="consts", bufs=1))
        persist = ctx.enter_context(tc.tile_pool(name="persist", bufs=1))
        sb = ctx.enter_context(tc.tile_pool(name="sb", bufs=2))
        keyp = ctx.enter_context(tc.tile_pool(name="keyp", bufs=2))
        gath = ctx.enter_context(tc.tile_pool(name="gath", bufs=2))
        mp = ctx.enter_context(tc.tile_pool(name="mp", bufs=1))
        psb = ctx.enter_context(tc.tile_pool(name="psb", bufs=3, space="PSUM"))
        pss = ctx.enter_context(tc.tile_pool(name="pss", bufs=2, space="PSUM"))

        # ---- DRAM I/O ----
        x_d = dram.tile([N, 3], F32, kind="ExternalInput", uniquify=False, name="x")
        win = {}
        for li, (C, O) in enumerate(LAYERS, start=1):
            win[f'wnt{li}'] = dram.tile([C, O], F32, kind="ExternalInput", uniquify=False, name=f"wnt{li}")
            win[f'wdt{li}'] = dram.tile([C, O], F32, kind="ExternalInput", uniquify=False, name=f"wdt{li}")
            win[f'bs{li}'] = dram.tile([1, O], F32, kind="ExternalInput", uniquify=False, name=f"bs{li}")
        w5t_d = dram.tile([512, 512], F32, kind="ExternalInput", uniquify=False, name="w5t")
        b5_d = dram.tile([1, 512], F32, kind="ExternalInput", uniquify=False, name="b5")
        wfc1_d = dram.tile([512, 256], F32, kind="ExternalInput", uniquify=False, name="wfc1")
        bfc1_d = dram.tile([128, 2], F32, kind="ExternalInput", uniquify=False, name="bfc1")
        wfc2_d = dram.tile([256, 128], F32, kind="ExternalInput", uniquify=False, name="wfc2")
        bfc2_d = dram.tile([128, 1], F32, kind="ExternalInput", uniquify=False, name="bfc2")
        wfc3_d = dram.tile([128, 40], F32, kind="ExternalInput", uniquify=False, name="wfc3")
        bfc3_d = dram.tile([1, 40], F32, kind="ExternalInput", uniquify=False, name="bfc3")
        out_d = dram.tile([40, 1], F32, kind="ExternalOutput", uniquify=False, name="out")
        dbg_d = None
        if stage != 'full':
            dbg_d = dram.tile([128, N], F32, kind="ExternalOutput", uniquify=False, name="dbg")

        a_ds = {li: dram.tile([N, O], F16 if li == 4 else F32, name=f"a_d{li}")
                for li, (C, O) in enumerate(LAYERS, start=1)}
        jw_ds = {li: dram.tile([N * KNN // 16, 128], I16, name=f"jw_d{li}")
                 for li in range(1, 5)}

        # ---- consts ----
        iotaJ = consts.tile([128, N], U32, tag="iotaJ")
        nc.gpsimd.iota(iotaJ[:, :], [[1, N]], base=0, channel_multiplier=0)
        ident = consts.tile([128, 128], F32, tag="ident")
        make_identity(nc, ident[:, :])
        onescol = consts.tile([128, 1], F32, tag="onescol")
        nc.vector.memset(onescol[:, :], 1.0)
        onesrow = consts.tile([1, N], F32, tag="onesrow")
        nc.vector.memset(onesrow[:, :], 1.0)
        negones = consts.tile([1, N], F32, tag="negones")
        nc.vector.memset(negones[:, :], -1.0)

        # persistent feature tensors
        x0T = persist.tile([3, N], F32, tag="x0T")
        x1T = persist.tile([64, N], F32, tag="x1T")
        x2T = persist.tile([64, N], F32, tag="x2T")
        x3T = persist.tile([128, N], F32, tag="x3T")
        x4Ta = persist.tile([128, N], F32, tag="x4Ta")
        x4Tb = persist.tile([128, N], F32, tag="x4Tb")

        # load x transposed: x_d is (N, 3) row-major
        xap = x_d[:, :]
        nc.sync.dma_start(
            x0T[:, :], bass.AP(xap.tensor, xap.offset, [[1, 3], [3, N]]))
        if stage == 'xload':
            nc.sync.dma_start(dbg_d[0:3, :], x0T[:, :])
            nc.sync.dma_start(out_d[:, :], onescol[0:40, :])
            return

        def edge_layer(li, xT, C, O, out_parts, post_tiles=None):
            a_d = a_ds[li]
            jw_d = jw_ds[li]
            wnt = sb.tile([C, O], F32, tag="wnt")
            wdt = sb.tile([C, O], F32, tag="wdt")
            bs = sb.tile([1, O], F32, tag="bs")
            nc.sync.dma_start(wnt[:, :], win[f'wnt{li}'][:, :])
            nc.sync.dma_start(wdt[:, :], win[f'wdt{li}'][:, :])
            nc.sync.dma_start(bs[:, :], win[f'bs{li}'][:, :])

            jf = sb.tile([128, NT * KNN], F32, tag="jf")
            xsq = sb.tile([C, N], F32, tag="xsq")
            nc.scalar.activation(xsq[:, :], xT[:, :], AF.Square)
            x2d = sb.tile([C, N], F32, tag="x2d")
            nc.scalar.activation(x2d[:, :], xT[:, :], AF.Copy, bias=0.0, scale=2.0)

            SO2 = mp.tile([2, N], F32, tag="SO2")
            OS2 = mp.tile([2, N], F32, tag="OS2")
            negS = mp.tile([1, N], F32, tag="negS")
            nc.vector.memset(OS2[0:1, :], -1.0)
            for h in range(2):
                cols = slice(h * 512, (h + 1) * 512)
                S_ps = pss.tile([1, 512], F32, tag="a")
                nc.tensor.matmul(
                    S_ps[:, :], lhsT=onescol[0:C, :], rhs=xsq[:, cols],
                    start=True, stop=True, skip_group_check=True)
                nc.scalar.activation(
                    SO2[0:1, cols], S_ps[:, :], AF.Copy, bias=0.0, scale=1.0)
                nc.scalar.activation(
                    negS[0:1, cols], S_ps[:, :], AF.Copy, bias=0.0, scale=-1.0)
            nc.sync.dma_start(SO2[1:2, :], onesrow[0:1, :])
            nc.sync.dma_start(OS2[1:2, :], negS[0:1, :])

            # a-rows to DRAM first so gathers can start as soon as idx ready
            for t in range(NT):
                a_ps = pss.tile([128, O], F32, tag="a")
                nc.tensor.matmul(
                    a_ps[:, :], lhsT=xT[:, t * 128:(t + 1) * 128], rhs=wnt[:, :],
                    start=True, stop=True, skip_group_check=True)
                a_sb = sb.tile([128, O], F16 if li == 4 else F32, tag="a_sb")
                nc.scalar.activation(a_sb[:, :], a_ps[:, :], AF.Copy)
                nc.sync.dma_start(a_d[t * 128:(t + 1) * 128, :], a_sb[:, :])

            m = mp.tile([128, NT, O], F32, tag="m")
            jwap = jw_d[:, :]
            pending_g = []
            ntl = NT
            if stage.startswith('topk1_'):
                ntl = int(stage.split('_')[1])
            for t in range(ntl):
                if stage in ('keys', 'pack', 'max1', 'mr1', 'ext', 'topkt0') and t > 0:
                    break
                tcols = slice(t * 128, (t + 1) * 128)
                kp = psb.tile([128, N], F32, tag="big")
                for h in range(2):
                    cols = slice(h * 512, (h + 1) * 512)
                    nc.tensor.matmul(
                        kp[:, cols], lhsT=xT[:, tcols], rhs=x2d[:, cols],
                        start=True, stop=False, skip_group_check=True)
                    nc.tensor.matmul(
                        kp[:, cols], lhsT=SO2[:, tcols], rhs=OS2[:, cols],
                        start=False, stop=True, skip_group_check=True)
                if stage == 'keys' and t == 0:
                    kcp = sb.tile([128, N], F32, tag="kcp")
                    nc.scalar.activation(kcp[:, :], kp[:, :], AF.Copy)
                    nc.sync.dma_start(dbg_d[:, :], kcp[:, :])
                    nc.sync.dma_start(out_d[:, :], onescol[0:40, :])
                    return 'stop'
                kb = keyp.tile([128, N], U32, tag="keysP")
                nc.vector.tensor_scalar(
                    kb[:, :], kp[:, :].bitcast(U32), 0xFFFFFC00, None,
                    op0=ALU.bitwise_and)
                nc.vector.tensor_tensor(
                    out=kb[:, :], in0=kb[:, :], in1=iotaJ[:, :], op=ALU.bitwise_or)
                if stage == 'pack' and t == 0:
                    kcp = sb.tile([128, N], F32, tag="kcp")
                    nc.vector.tensor_copy(kcp[:, :].bitcast(U32), kb[:, :])
                    nc.sync.dma_start(dbg_d[:, :], kcp[:, :])
                    nc.sync.dma_start(out_d[:, :], onescol[0:40, :])
                    return 'stop'
                kbf = kb[:, :].bitcast(F32)
                v24 = sb.tile([128, 24], F32, tag="v24")
                nc.vector.max(v24[:, 0:8], kbf)
                if stage == 'max1' and t == 0:
                    nc.sync.dma_start(dbg_d[:, 0:8], v24[:, 0:8])
                    nc.sync.dma_start(out_d[:, :], onescol[0:40, :])
                    return 'stop'
                nc.vector.match_replace(kbf, v24[:, 0:8], kbf, NEG_BIG)
                if stage == 'mr1' and t == 0:
                    nc.sync.dma_start(dbg_d[:, :], kb[:, :].bitcast(F32))
                    nc.sync.dma_start(out_d[:, :], onescol[0:40, :])
                    return 'stop'
                nc.vector.max(v24[:, 8:16], kbf)
                nc.vector.match_replace(kbf, v24[:, 8:16], kbf, NEG_BIG)
                nc.vector.max(v24[:, 16:24], kbf)
                if stage == 'ext' and t == 0:
                    nc.sync.dma_start(dbg_d[:, 0:24], v24[:, :])
                    nc.sync.dma_start(out_d[:, :], onescol[0:40, :])
                    return 'stop'
                j20 = sb.tile([128, KNN], U32, tag="j20")
                nc.vector.tensor_scalar(
                    j20[:, :], v24[:, 0:KNN].bitcast(U32), 0x3FF, None,
                    op0=ALU.bitwise_and)
                # j as fp32 values, accumulated for PE transpose
                nc.vector.tensor_copy(jf[:, t * KNN:(t + 1) * KNN], j20[:, :])
                if stage == 'topkt0' and t == 0:
                    nc.sync.dma_start(dbg_d[:, 0:KNN], jf[:, 0:KNN])
                    nc.sync.dma_start(out_d[:, :], onescol[0:40, :])
                    return 'stop'

                pend = []
                if t >= 3 and t % 2 == 1:
                    pend.append((t - 3) // 2)
                if t == ntl - 1:
                    pend.append(3)
                for tp in pend:
                    jT_ps = pss.tile([KNN, 256], F32, tag="a")
                    for tl in range(2):
                        tt = 2 * tp + tl
                        nc.tensor.matmul(
                            jT_ps[:, tl * 128:(tl + 1) * 128],
                            lhsT=jf[:, tt * KNN:(tt + 1) * KNN], rhs=ident[:, 0:128],
                            is_transpose=True, start=(tl == 0), stop=(tl == 1),
                            skip_group_check=True)
                    jTi = sb.tile([KNN, 256], I16, tag="jTi")
                    nc.vector.tensor_copy(jTi[:, :], jT_ps[:, :])
                    for tl in range(2):
                        tt = 2 * tp + tl
                        dst = bass.AP(jwap.tensor, jwap.offset + tt * 160 * 128,
                                      [[1024, KNN], [128, 8], [1, 16]])
                        nc.sync.dma_start(
                            dst, jTi[:, tl * 128:(tl + 1) * 128].rearrange(
                                "k (h s) -> k h s", s=16))
                    src_ap = bass.AP(jwap.tensor, jwap.offset + tp * 320 * 128,
                                     [[128, 320], [1, 128]])
                    idq = keyp.tile([128, 16 * KNN], I16, tag="idxq", bufs=4)
                    nc.scalar.dma_start_transpose(idq[:, :], src_ap)
                    for rr in range(1, 8):
                        nc.scalar.dma_start(
                            idq[16 * rr:16 * (rr + 1), :], idq[0:16, :])

                    gdt = F16 if li == 4 else F32
                    g = gath.tile([128, 2 * KNN, O], gdt, tag="g", bufs=3)
                    nc.gpsimd.dma_gather(
                        out_ap=g[:, :, :], in_ap=a_d[:, :],
                        idxs_ap=idq[:, :],
                        num_idxs=KNN * 256, num_idxs_reg=KNN * 256, elem_size=O,
                        single_packet=False)
                    pending_g.append((tp, g))



            if stage.startswith('topk1'):
                nc.sync.dma_start(dbg_d[0:128, 0:NT * KNN], jf[:, :])
                nc.sync.dma_start(out_d[:, :], onescol[0:40, :])
                return 'stop'

            if post_tiles is not None:
                post_tiles()
            for tp, g in pending_g:
                gap = g[:, :, :]
                red_in = bass.AP(
                    gap.tensor, gap.offset,
                    [gap.ap[0], [KNN * O, 2], [1, O], [O, KNN]])
                nc.vector.tensor_reduce(
                    out=m[:, 2 * tp:2 * tp + 2, :], in_=red_in,
                    axis=AX.X, op=ALU.max)

            # transpose m + c matmul + lrelu -> out_parts
            for ot, (op_ap, orow) in enumerate(out_parts):
                px = psb.tile([orow, N], F32, tag="big")
                for t in range(NT):
                    nc.tensor.matmul(
                        px[:, t * 128:(t + 1) * 128],
                        lhsT=m[:, t, ot * 128:ot * 128 + orow],
                        rhs=ident[:, 0:128],
                        is_transpose=True, start=(t % 4 == 0), stop=False,
                        skip_group_check=True)
                for h in range(2):
                    cols = slice(h * 512, (h + 1) * 512)
                    nc.tensor.matmul(
                        px[:, cols],
                        lhsT=wdt[:, ot * 128:ot * 128 + orow],
                        rhs=xT[:, cols],
                        start=False, stop=False, skip_group_check=True)
                    nc.tensor.matmul(
                        px[:, cols],
                        lhsT=bs[0:1, ot * 128:ot * 128 + orow],
                        rhs=onesrow[0:1, cols],
                        start=False, stop=True, skip_group_check=True)
                nc.scalar.activation(op_ap, px[:, :], AF.Prelu, alpha=NEG_SLOPE)

        w5sb = {}
        for ci, (rows, k0) in enumerate([(64, 0), (64, 64), (128, 128),
                                         (128, 256), (128, 384)]):
            w5c = consts.tile([rows, 512], F32, tag=f"w5c{ci}")
            nc.sync.dma_start(w5c[:, :], w5t_d[k0:k0 + rows, :])
            w5sb[ci] = w5c
        b5sb = consts.tile([1, 512], F32, tag="b5sb")
        nc.sync.dma_start(b5sb[:, :], b5_d[:, :])
        zpart = persist.tile([128, NT, 512], F32, tag="zpart")

        def zpart_fill():
            for t in range(NT):
                tcols = slice(t * 128, (t + 1) * 128)
                zp_ps = pss.tile([128, 512], F32, tag="a")
                for ci, (xt, rows) in enumerate(
                        [(x1T, 64), (x2T, 64), (x3T, 128)]):
                    nc.tensor.matmul(
                        zp_ps[:, :], lhsT=xt[:, tcols], rhs=w5sb[ci][:, :],
                        start=(ci == 0), stop=(ci == 2), skip_group_check=True)
                nc.scalar.activation(zpart[:, t, :], zp_ps[:, :], AF.Copy)

        r = edge_layer(1, x0T, 3, 64, [(x1T[:, :], 64)])
        if r == 'stop':
            return
        if stage == 'gath1':
            nc.sync.dma_start(dbg_d[0:64, :], x1T[:, :])
            nc.sync.dma_start(out_d[:, :], onescol[0:40, :])
            return
        edge_layer(2, x1T, 64, 64, [(x2T[:, :], 64)])
        edge_layer(3, x2T, 64, 128, [(x3T[:, :], 128)])
        edge_layer(4, x3T, 128, 256, [(x4Ta[:, :], 128), (x4Tb[:, :], 128)],
                   post_tiles=zpart_fill)

        # ---- head: conv5 (x4 chunks; x1-x3 partials precomputed) + max pool ----
        zmax = persist.tile([128, 512], F32, tag="zmax")
        for t in range(NT):
            tcols = slice(t * 128, (t + 1) * 128)
            z_ps = pss.tile([128, 512], F32, tag="a")
            for ci, (xt, rows, k0) in enumerate(
                    [(x4Ta, 128, 256), (x4Tb, 128, 384)]):
                nc.tensor.matmul(
                    z_ps[:, :], lhsT=xt[:, tcols], rhs=w5sb[3 + ci][:, :],
                    start=(ci == 0), stop=False, skip_group_check=True)
            nc.tensor.matmul(
                z_ps[:, :], lhsT=onesrow[0:1, tcols],
                rhs=b5sb[:, :], start=False, stop=True, skip_group_check=True)
            zsb = sb.tile([128, 512], F32, tag="zsb")
            nc.vector.tensor_tensor(
                out=zsb[:, :], in0=zpart[:, t, :], in1=z_ps[:, :], op=ALU.add)
            if t == 0:
                nc.scalar.activation(zmax[:, :], zsb[:, :], AF.Copy)
            else:
                nc.vector.tensor_tensor(
                    out=zmax[:, :], in0=zmax[:, :], in1=zsb[:, :], op=ALU.max)
        # partition tree-max 128 -> 1... then we need yT [128, 4] instead:
        # transpose zmax chunks and reduce along free dim.
        yT = persist.tile([128, 4], F32, tag="yT")
        for cchunk in range(4):
            zt_ps = pss.tile([128, 128], F32, tag="a")
            nc.tensor.matmul(
                zt_ps[:, :], lhsT=zmax[:, cchunk * 128:(cchunk + 1) * 128],
                rhs=ident[:, 0:128], is_transpose=True, start=True, stop=True,
                skip_group_check=True)
            nc.vector.tensor_reduce(
                out=yT[:, cchunk:cchunk + 1], in_=zt_ps[:, :],
                axis=AX.X, op=ALU.max)
        # leaky relu on yT
        yTr = persist.tile([128, 4], F32, tag="yTr")
        nc.scalar.activation(yTr[:, :], yT[:, :], AF.Prelu, alpha=NEG_SLOPE)

        # ---- FC head ----
        wfc1sb = consts.tile([128, 4, 256], F32, tag="wfc1sb")
        for c in range(4):
            nc.sync.dma_start(wfc1sb[:, c, :], wfc1_d[c * 128:(c + 1) * 128, :])
        bfc1sb = consts.tile([128, 2], F32, tag="bfc1sb")
        nc.sync.dma_start(bfc1sb[:, :], bfc1_d[:, :])
        wfc2sb = consts.tile([128, 2, 128], F32, tag="wfc2sb")
        for c in range(2):
            nc.sync.dma_start(wfc2sb[:, c, :], wfc2_d[c * 128:(c + 1) * 128, :])
        bfc2sb = consts.tile([128, 1], F32, tag="bfc2sb")
        nc.sync.dma_start(bfc2sb[:, :], bfc2_d[:, :])
        wfc3sb = consts.tile([128, 40], F32, tag="wfc3sb")
        nc.sync.dma_start(wfc3sb[:, :], wfc3_d[:, :])
        bfc3sb = consts.tile([1, 40], F32, tag="bfc3sb")
        nc.sync.dma_start(bfc3sb[:, :], bfc3_d[:, :])

        h1sb = persist.tile([128, 2], F32, tag="h1sb")
        for mt in range(2):
            h1_ps = pss.tile([128, 1], F32, tag="a")
            for c in range(4):
                nc.tensor.matmul(
                    h1_ps[:, :], lhsT=wfc1sb[:, c, mt * 128:(mt + 1) * 128],
                    rhs=yTr[:, c:c + 1],
                    start=(c == 0), stop=(c == 3), skip_group_check=True)
            nc.scalar.activation(
                h1sb[:, mt:mt + 1], h1_ps[:, :], AF.Prelu,
                bias=bfc1sb[:, mt:mt + 1], scale=1.0, alpha=NEG_SLOPE)
        h2sb = persist.tile([128, 1], F32, tag="h2sb")
        h2_ps = pss.tile([128, 1], F32, tag="a")
        for c in range(2):
            nc.tensor.matmul(
                h2_ps[:, :], lhsT=wfc2sb[:, c, :], rhs=h1sb[:, c:c + 1],
                start=(c == 0), stop=(c == 1), skip_group_check=True)
        nc.scalar.activation(
            h2sb[:, :], h2_ps[:, :], AF.Prelu,
            bias=bfc2sb[:, :], scale=1.0, alpha=NEG_SLOPE)

        out_ps = pss.tile([40, 1], F32, tag="a")
        nc.tensor.matmul(
            out_ps[:, :], lhsT=wfc3sb[:, :], rhs=h2sb[:, :],
            start=True, stop=False, skip_group_check=True)
        nc.tensor.matmul(
            out_ps[:, :], lhsT=bfc3sb[:, :], rhs=onescol[0:1, :],
            start=False, stop=True, skip_group_check=True)
        out_sb = persist.tile([40, 1], F32, tag="out_sb")
        nc.scalar.activation(out_sb[:, :], out_ps[:, :], AF.Copy)
        nc.sync.dma_start(out_d[:, :], out_sb[:, :])


# ---------------------------------------------------------------------------
# harness entry point
# ---------------------------------------------------------------------------
_NC_CACHE = {}


def _get_nc():
    if 'nc' not in _NC_CACHE:
        _NC_CACHE['nc'] = build_nc()
    return _NC_CACHE['nc']


def kernel(**inputs):
    """Full-batch EdgeCNN forward. x: (8, 1024, 3) -> (8, 40) float32.

    Pure data parallel: batch element b runs on NeuronCore b.
    """
    from concourse.bass_utils import run_bass_kernel_spmd

    inp = {k: np.asarray(v) for k, v in inputs.items()}
    prep = host_prep(inp)
    nc = _get_nc()
    in_maps = []
    for b in range(8):
        m = {'x': np.ascontiguousarray(inp['x'][b]).astype(np.float32)}
        m.update(prep)
        in_maps.append(m)
    res = run_bass_kernel_spmd(nc, in_maps, core_ids=list(range(8)))
    out = np.stack([res.results[b]['out'].reshape(40) for b in range(8)])
    return out.astype(np.float32)



# revision 4
# speedup vs baseline: 1.1709x; 1.1709x over previous
"""EdgeCNN (DGCNN) Bass/Tile kernel for TRN2 — one batch element per core.

Per edge-conv layer (N=1024 points, K=20 neighbors):
  1. PE: packed-key matmul  pd[n,j] = 2<xn,xj> - S[j] - S[n]   (PSUM, fp32)
  2. DVE: one-pass (pd & ~0x3FF) | j  -> packed keys (scalar_tensor_tensor)
  3. DVE: 3x max8 + 2x match_replace -> top-20 packed keys; extract j
  4. idx -> DRAM -> read back in dma_gather wrapped layout (partition = n%16)
  5. SWDGE dma_gather (4 queues round-robin; one Q7 core-pair per queue) of
     rows of a = x @ (g~ Wn)^T; DVE strided reduce_max over k
  6. PE: transpose(m) + c-matmul (c = x @ (g~(Wc-Wn))^T + b) accumulated in PSUM
  7. ACT: leaky-relu (Prelu alpha=0.2) PSUM -> next layer xT
Head: conv5 via K-chunk accumulation, global max-pool, 3 FC layers on PE.
"""

import contextlib

import numpy as np

import concourse.bass as bass
import concourse.bacc as bacc
import concourse.mybir as mybir
from concourse.tile import TileContext
from concourse.masks import make_identity

F32 = mybir.dt.float32
U32 = mybir.dt.uint32
I16 = mybir.dt.int16
F16 = mybir.dt.float16
AF = mybir.ActivationFunctionType
ALU = mybir.AluOpType
AX = mybir.AxisListType

N = 1024
KNN = 20
NT = 8
NEG_SLOPE = 0.2
BNI = np.float32(1.0 / np.sqrt(1.0 + 1e-5))
LAYERS = [(3, 64), (64, 64), (64, 128), (128, 256)]
NEG_BIG = -3.0e38
NQ = 4  # SWDGE queues


def host_prep(inp):
    """Fold BN scale/bias into weights; transpose for device layout."""
    d = {}
    for li, (C, O) in enumerate(LAYERS, start=1):
        W = inp[f'W{li}'].astype(np.float32)
        g = inp[f'g{li}'].astype(np.float32)
        b = inp[f'b{li}'].astype(np.float32)
        gt = g * BNI
        Wn = W[:, :C]
        Wc = W[:, C:]
        d[f'wnt{li}'] = np.ascontiguousarray((gt[:, None] * Wn).T)          # (C, O)
        d[f'wdt{li}'] = np.ascontiguousarray((gt[:, None] * (Wc - Wn)).T)   # (C, O)
        d[f'bs{li}'] = b.reshape(1, O).copy()
    g5 = inp['g5'].astype(np.float32) * BNI
    d['w5t'] = np.ascontiguousarray((g5[:, None] * inp['W5']).T)            # (512, 512)
    d['b5'] = inp['b5'].reshape(1, 512).astype(np.float32).copy()
    g1 = inp['bng1'].astype(np.float32) * BNI
    d['wfc1'] = np.ascontiguousarray((g1[:, None] * inp['fc1_w']).T)        # (512, 256)
    bf1 = g1 * inp['fc1_b'].astype(np.float32) + inp['bnb1'].astype(np.float32)
    d['bfc1'] = np.ascontiguousarray(bf1.reshape(2, 128).T)                 # (128, 2)
    g2 = inp['bng2'].astype(np.float32) * BNI
    d['wfc2'] = np.ascontiguousarray((g2[:, None] * inp['fc2_w']).T)        # (256, 128)
    bf2 = g2 * inp['fc2_b'].astype(np.float32) + inp['bnb2'].astype(np.float32)
    d['bfc2'] = np.ascontiguousarray(bf2.reshape(128, 1))                   # (128, 1)
    d['wfc3'] = np.ascontiguousarray(inp['fc3_w'].T)                        # (128, 40)
    d['bfc3'] = inp['fc3_b'].reshape(1, 40).astype(np.float32).copy()
    return d


def build_nc(stage='full'):
    nc = bacc.Bacc("TRN2", target_bir_lowering=False, debug=False, num_devices=8,
                   num_swdge_queues=NQ)
    with TileContext(nc) as tc:
        _trace(nc, tc, stage)
    nc.compile()
    return nc


def _trace(nc, tc, stage='full'):
    with contextlib.ExitStack() as ctx:
        dram = ctx.enter_context(tc.tile_pool(name="dram", bufs=1, space="DRAM"))
        consts = ctx.enter_context(tc.tile_pool(name="consts", bufs=1))
        persist = ctx.enter_context(tc.tile_pool(name="persist", bufs=1))
        sb = ctx.enter_context(tc.tile_pool(name="sb", bufs=2))
        keyp = ctx.enter_context(tc.tile_pool(name="keyp", bufs=2))
        gath = ctx.enter_context(tc.tile_pool(name="gath", bufs=3))
        mp = ctx.enter_context(tc.tile_pool(name="mp", bufs=1))
        psb = ctx.enter_context(tc.tile_pool(name="psb", bufs=3, space="PSUM"))
        pss = ctx.enter_context(tc.tile_pool(name="pss", bufs=2, space="PSUM"))

        # ---- DRAM I/O ----
        x_d = dram.tile([N, 3], F32, kind="ExternalInput", uniquify=False, name="x")
        win = {}
        for li, (C, O) in enumerate(LAYERS, start=1):
            win[f'wnt{li}'] = dram.tile([C, O], F32, kind="ExternalInput", uniquify=False, name=f"wnt{li}")
            win[f'wdt{li}'] = dram.tile([C, O], F32, kind="ExternalInput", uniquify=False, name=f"wdt{li}")
            win[f'bs{li}'] = dram.tile([1, O], F32, kind="ExternalInput", uniquify=False, name=f"bs{li}")
        w5t_d = dram.tile([512, 512], F32, kind="ExternalInput", uniquify=False, name="w5t")
        b5_d = dram.tile([1, 512], F32, kind="ExternalInput", uniquify=False, name="b5")
        wfc1_d = dram.tile([512, 256], F32, kind="ExternalInput", uniquify=False, name="wfc1")
        bfc1_d = dram.tile([128, 2], F32, kind="ExternalInput", uniquify=False, name="bfc1")
        wfc2_d = dram.tile([256, 128], F32, kind="ExternalInput", uniquify=False, name="wfc2")
        bfc2_d = dram.tile([128, 1], F32, kind="ExternalInput", uniquify=False, name="bfc2")
        wfc3_d = dram.tile([128, 40], F32, kind="ExternalInput", uniquify=False, name="wfc3")
        bfc3_d = dram.tile([1, 40], F32, kind="ExternalInput", uniquify=False, name="bfc3")
        out_d = dram.tile([40, 1], F32, kind="ExternalOutput", uniquify=False, name="out")
        dbg_d = None
        if stage != 'full':
            dbg_d = dram.tile([128, N], F32, kind="ExternalOutput", uniquify=False, name="dbg")

        a_ds = {li: dram.tile([N, O], F16 if li == 4 else F32, name=f"a_d{li}")
                for li, (C, O) in enumerate(LAYERS, start=1)}
        jw_ds = {li: dram.tile([N * KNN // 16, 128], I16, name=f"jw_d{li}")
                 for li in range(1, 5)}

        # ---- consts ----
        iotaJ = consts.tile([128, N], U32, tag="iotaJ")
        nc.gpsimd.iota(iotaJ[:, :], [[1, N]], base=0, channel_multiplier=0)
        ident = consts.tile([128, 128], F32, tag="ident")
        make_identity(nc, ident[:, :])
        onescol = consts.tile([128, 1], F32, tag="onescol")
        nc.vector.memset(onescol[:, :], 1.0)
        onesrow = consts.tile([1, N], F32, tag="onesrow")
        nc.vector.memset(onesrow[:, :], 1.0)
        negones = consts.tile([1, N], F32, tag="negones")
        nc.vector.memset(negones[:, :], -1.0)
        maskc = consts.tile([128, 1], U32, tag="maskc")
        nc.vector.memset(maskc[:, :], 0xFFFFFC00)

        # persistent feature tensors
        x0T = persist.tile([3, N], F32, tag="x0T")
        x1T = persist.tile([64, N], F32, tag="x1T")
        x2T = persist.tile([64, N], F32, tag="x2T")
        x3T = persist.tile([128, N], F32, tag="x3T")
        x4Ta = persist.tile([128, N], F32, tag="x4Ta")
        x4Tb = persist.tile([128, N], F32, tag="x4Tb")

        # load x transposed: x_d is (N, 3) row-major
        xap = x_d[:, :]
        nc.sync.dma_start(
            x0T[:, :], bass.AP(xap.tensor, xap.offset, [[1, 3], [3, N]]))
        if stage == 'xload':
            nc.sync.dma_start(dbg_d[0:3, :], x0T[:, :])
            nc.sync.dma_start(out_d[:, :], onescol[0:40, :])
            return

        def edge_layer(li, xT, C, O, out_parts, post_tiles=None):
            a_d = a_ds[li]
            jw_d = jw_ds[li]
            jwap = jw_d[:, :]
            wnt = sb.tile([C, O], F32, tag="wnt")
            wdt = sb.tile([C, O], F32, tag="wdt")
            bs = sb.tile([1, O], F32, tag="bs")
            nc.sync.dma_start(wnt[:, :], win[f'wnt{li}'][:, :])
            nc.sync.dma_start(wdt[:, :], win[f'wdt{li}'][:, :])
            nc.sync.dma_start(bs[:, :], win[f'bs{li}'][:, :])

            jf = sb.tile([128, NT * KNN], F32, tag="jf")
            xsq = sb.tile([C, N], F32, tag="xsq")
            nc.scalar.activation(xsq[:, :], xT[:, :], AF.Square)
            x2d = sb.tile([C, N], F32, tag="x2d")
            nc.scalar.activation(x2d[:, :], xT[:, :], AF.Copy, bias=0.0, scale=2.0)

            SO2 = mp.tile([2, N], F32, tag="SO2")
            OS2 = mp.tile([2, N], F32, tag="OS2")
            negS = mp.tile([1, N], F32, tag="negS")
            nc.vector.memset(OS2[0:1, :], -1.0)
            for h in range(2):
                cols = slice(h * 512, (h + 1) * 512)
                S_ps = pss.tile([1, 512], F32, tag="a")
                nc.tensor.matmul(
                    S_ps[:, :], lhsT=onescol[0:C, :], rhs=xsq[:, cols],
                    start=True, stop=True, skip_group_check=True)
                nc.scalar.activation(
                    SO2[0:1, cols], S_ps[:, :], AF.Copy, bias=0.0, scale=1.0)
                nc.scalar.activation(
                    negS[0:1, cols], S_ps[:, :], AF.Copy, bias=0.0, scale=-1.0)
            nc.sync.dma_start(SO2[1:2, :], onesrow[0:1, :])
            nc.sync.dma_start(OS2[1:2, :], negS[0:1, :])

            # a-rows to DRAM first so gathers can start as soon as idx ready
            for t in range(NT):
                a_ps = pss.tile([128, O], F32, tag="a")
                nc.tensor.matmul(
                    a_ps[:, :], lhsT=xT[:, t * 128:(t + 1) * 128], rhs=wnt[:, :],
                    start=True, stop=True, skip_group_check=True)
                a_sb = sb.tile([128, O], F16 if li == 4 else F32, tag="a_sb")
                nc.scalar.activation(a_sb[:, :], a_ps[:, :], AF.Copy)
                nc.sync.dma_start(a_d[t * 128:(t + 1) * 128, :], a_sb[:, :])

            m = mp.tile([128, NT, O], F32, tag="m")
            ntl = NT
            if stage.startswith('topk1_'):
                ntl = int(stage.split('_')[1])
            for t in range(ntl):
                if stage in ('keys', 'pack', 'max1', 'mr1', 'ext', 'topkt0') and t > 0:
                    break
                tcols = slice(t * 128, (t + 1) * 128)
                kp = psb.tile([128, N], F32, tag="big")
                for h in range(2):
                    cols = slice(h * 512, (h + 1) * 512)
                    nc.tensor.matmul(
                        kp[:, cols], lhsT=xT[:, tcols], rhs=x2d[:, cols],
                        start=True, stop=False, skip_group_check=True)
                    nc.tensor.matmul(
                        kp[:, cols], lhsT=SO2[:, tcols], rhs=OS2[:, cols],
                        start=False, stop=True, skip_group_check=True)
                if stage == 'keys' and t == 0:
                    kcp = sb.tile([128, N], F32, tag="kcp")
                    nc.scalar.activation(kcp[:, :], kp[:, :], AF.Copy)
                    nc.sync.dma_start(dbg_d[:, :], kcp[:, :])
                    nc.sync.dma_start(out_d[:, :], onescol[0:40, :])
                    return 'stop'
                kb = keyp.tile([128, N], U32, tag="keysP")
                nc.vector.scalar_tensor_tensor(
                    out=kb[:, :], in0=kp[:, :].bitcast(U32), scalar=maskc[:, 0:1],
                    in1=iotaJ[:, :], op0=ALU.bitwise_and, op1=ALU.bitwise_or)
                if stage == 'pack' and t == 0:
                    kcp = sb.tile([128, N], F32, tag="kcp")
                    nc.vector.tensor_copy(kcp[:, :].bitcast(U32), kb[:, :])
                    nc.sync.dma_start(dbg_d[:, :], kcp[:, :])
                    nc.sync.dma_start(out_d[:, :], onescol[0:40, :])
                    return 'stop'
                kbf = kb[:, :].bitcast(F32)
                v24 = sb.tile([128, 24], F32, tag="v24")
                nc.vector.max(v24[:, 0:8], kbf)
                if stage == 'max1' and t == 0:
                    nc.sync.dma_start(dbg_d[:, 0:8], v24[:, 0:8])
                    nc.sync.dma_start(out_d[:, :], onescol[0:40, :])
                    return 'stop'
                nc.vector.match_replace(kbf, v24[:, 0:8], kbf, NEG_BIG)
                if stage == 'mr1' and t == 0:
                    nc.sync.dma_start(dbg_d[:, :], kb[:, :].bitcast(F32))
                    nc.sync.dma_start(out_d[:, :], onescol[0:40, :])
                    return 'stop'
                nc.vector.max(v24[:, 8:16], kbf)
                nc.vector.match_replace(kbf, v24[:, 8:16], kbf, NEG_BIG)
                nc.vector.max(v24[:, 16:24], kbf)
                if stage == 'ext' and t == 0:
                    nc.sync.dma_start(dbg_d[:, 0:24], v24[:, :])
                    nc.sync.dma_start(out_d[:, :], onescol[0:40, :])
                    return 'stop'
                j20 = sb.tile([128, KNN], U32, tag="j20")
                nc.vector.tensor_scalar(
                    j20[:, :], v24[:, 0:KNN].bitcast(U32), 0x3FF, None,
                    op0=ALU.bitwise_and)
                # j as fp32 values, accumulated for PE transpose
                nc.vector.tensor_copy(jf[:, t * KNN:(t + 1) * KNN], j20[:, :])
                if stage == 'topkt0' and t == 0:
                    nc.sync.dma_start(dbg_d[:, 0:KNN], jf[:, 0:KNN])
                    nc.sync.dma_start(out_d[:, :], onescol[0:40, :])
                    return 'stop'

                # per-tile idx wrap + gather on queue t%NQ
                jT_ps = pss.tile([KNN, 128], F32, tag="a")
                nc.tensor.matmul(
                    jT_ps[:, :], lhsT=jf[:, t * KNN:(t + 1) * KNN],
                    rhs=ident[:, 0:128], is_transpose=True, start=True, stop=True,
                    skip_group_check=True)
                jTi = sb.tile([KNN, 128], I16, tag="jTi")
                nc.vector.tensor_copy(jTi[:, :], jT_ps[:, :])
                dst = bass.AP(jwap.tensor, jwap.offset + t * 160 * 128,
                              [[1024, KNN], [128, 8], [1, 16]])
                nc.sync.dma_start(
                    dst, jTi[:, :].rearrange("k (h s) -> k h s", s=16))
                src_ap = bass.AP(jwap.tensor, jwap.offset + t * 160 * 128,
                                 [[128, 160], [1, 128]])
                idq = keyp.tile([128, 10 * 16], I16, tag="idxq", bufs=4)
                nc.scalar.dma_start_transpose(idq[:, :], src_ap)
                for rr in range(1, 8):
                    nc.scalar.dma_start(
                        idq[16 * rr:16 * (rr + 1), :], idq[0:16, :])

                gdt = F16 if li == 4 else F32
                g = gath.tile([128, KNN, O], gdt, tag="g", bufs=3)
                nc.gpsimd.dma_gather(
                    out_ap=g[:, :, :], in_ap=a_d[:, :],
                    idxs_ap=idq[:, :],
                    num_idxs=KNN * 128, num_idxs_reg=KNN * 128, elem_size=O,
                    single_packet=False, queue_num=t % NQ)
                gap = g[:, :, :]
                red_in = bass.AP(
                    gap.tensor, gap.offset,
                    [gap.ap[0], [1, O], [O, KNN]])
                nc.vector.tensor_reduce(
                    out=m[:, t, :], in_=red_in, axis=AX.X, op=ALU.max)

            if stage.startswith('topk1'):
                nc.sync.dma_start(dbg_d[0:128, 0:NT * KNN], jf[:, :])
                nc.sync.dma_start(out_d[:, :], onescol[0:40, :])
                return 'stop'

            if post_tiles is not None:
                post_tiles()

            # transpose m + c matmul + lrelu -> out_parts
            for ot, (op_ap, orow) in enumerate(out_parts):
                px = psb.tile([orow, N], F32, tag="big")
                for t in range(NT):
                    nc.tensor.matmul(
                        px[:, t * 128:(t + 1) * 128],
                        lhsT=m[:, t, ot * 128:ot * 128 + orow],
                        rhs=ident[:, 0:128],
                        is_transpose=True, start=(t % 4 == 0), stop=False,
                        skip_group_check=True)
                for h in range(2):
                    cols = slice(h * 512, (h + 1) * 512)
                    nc.tensor.matmul(
                        px[:, cols],
                        lhsT=wdt[:, ot * 128:ot * 128 + orow],
                        rhs=xT[:, cols],
                        start=False, stop=False, skip_group_check=True)
                    nc.tensor.matmul(
                        px[:, cols],
                        lhsT=bs[0:1, ot * 128:ot * 128 + orow],
                        rhs=onesrow[0:1, cols],
                        start=False, stop=True, skip_group_check=True)
                nc.scalar.activation(op_ap, px[:, :], AF.Prelu, alpha=NEG_SLOPE)

        w5sb = {}
        for ci, (rows, k0) in enumerate([(64, 0), (64, 64), (128, 128),
                                         (128, 256), (128, 384)]):
            w5c = consts.tile([rows, 512], F32, tag=f"w5c{ci}")
            nc.sync.dma_start(w5c[:, :], w5t_d[k0:k0 + rows, :])
            w5sb[ci] = w5c
        b5sb = consts.tile([1, 512], F32, tag="b5sb")
        nc.sync.dma_start(b5sb[:, :], b5_d[:, :])
        zpart = persist.tile([128, NT, 512], F32, tag="zpart")

        def zpart_fill():
            for t in range(NT):
                tcols = slice(t * 128, (t + 1) * 128)
                zp_ps = pss.tile([128, 512], F32, tag="a")
                for ci, (xt, rows) in enumerate(
                        [(x1T, 64), (x2T, 64), (x3T, 128)]):
                    nc.tensor.matmul(
                        zp_ps[:, :], lhsT=xt[:, tcols], rhs=w5sb[ci][:, :],
                        start=(ci == 0), stop=(ci == 2), skip_group_check=True)
                nc.scalar.activation(zpart[:, t, :], zp_ps[:, :], AF.Copy)

        r = edge_layer(1, x0T, 3, 64, [(x1T[:, :], 64)])
        if r == 'stop':
            return
        if stage == 'gath1':
            nc.sync.dma_start(dbg_d[0:64, :], x1T[:, :])
            nc.sync.dma_start(out_d[:, :], onescol[0:40, :])
            return
        edge_layer(2, x1T, 64, 64, [(x2T[:, :], 64)])
        edge_layer(3, x2T, 64, 128, [(x3T[:, :], 128)])
        edge_layer(4, x3T, 128, 256, [(x4Ta[:, :], 128), (x4Tb[:, :], 128)],
                   post_tiles=zpart_fill)

        # ---- head: conv5 (x4 chunks; x1-x3 partials precomputed) + max pool ----
        zmax = persist.tile([128, 512], F32, tag="zmax")
        for t in range(NT):
            tcols = slice(t * 128, (t + 1) * 128)
            z_ps = pss.tile([128, 512], F32, tag="a")
            for ci, (xt, rows, k0) in enumerate(
                    [(x4Ta, 128, 256), (x4Tb, 128, 384)]):
                nc.tensor.matmul(
                    z_ps[:, :], lhsT=xt[:, tcols], rhs=w5sb[3 + ci][:, :],
                    start=(ci == 0), stop=False, skip_group_check=True)
            nc.tensor.matmul(
                z_ps[:, :], lhsT=onesrow[0:1, tcols],
                rhs=b5sb[:, :], start=False, stop=True, skip_group_check=True)
            zsb = sb.tile([128, 512], F32, tag="zsb")
            nc.vector.tensor_tensor(
                out=zsb[:, :], in0=zpart[:, t, :], in1=z_ps[:, :], op=ALU.add)
            if t == 0:
                nc.scalar.activation(zmax[:, :], zsb[:, :], AF.Copy)
            else:
                nc.vector.tensor_tensor(
                    out=zmax[:, :], in0=zmax[:, :], in1=zsb[:, :], op=ALU.max)
        # partition tree-max 128 -> 1... then we need yT [128, 4] instead:
        # transpose zmax chunks and reduce along free dim.
        yT = persist.tile([128, 4], F32, tag="yT")
        for cchunk in range(4):
            zt_ps = pss.tile([128, 128], F32, tag="a")
            nc.tensor.matmul(
                zt_ps[:, :], lhsT=zmax[:, cchunk * 128:(cchunk + 1) * 128],
                rhs=ident[:, 0:128], is_transpose=True, start=True, stop=True,
                skip_group_check=True)
            nc.vector.tensor_reduce(
                out=yT[:, cchunk:cchunk + 1], in_=zt_ps[:, :],
                axis=AX.X, op=ALU.max)
        # leaky relu on yT
        yTr = persist.tile([128, 4], F32, tag="yTr")
        nc.scalar.activation(yTr[:, :], yT[:, :], AF.Prelu, alpha=NEG_SLOPE)

        # ---- FC head ----
        wfc1sb = consts.tile([128, 4, 256], F32, tag="wfc1sb")
        for c in range(4):
            nc.sync.dma_start(wfc1sb[:, c, :], wfc1_d[c * 128:(c + 1) * 128, :])
        bfc1sb = consts.tile([128, 2], F32, tag="bfc1sb")
        nc.sync.dma_start(bfc1sb[:, :], bfc1_d[:, :])
        wfc2sb = consts.tile([128, 2, 128], F32, tag="wfc2sb")
        for c in range(2):
            nc.sync.dma_start(wfc2sb[:, c, :], wfc2_d[c * 128:(c + 1) * 128, :])
        bfc2sb = consts.tile([128, 1], F32, tag="bfc2sb")
        nc.sync.dma_start(bfc2sb[:, :], bfc2_d[:, :])
        wfc3sb = consts.tile([128, 40], F32, tag="wfc3sb")
        nc.sync.dma_start(wfc3sb[:, :], wfc3_d[:, :])
        bfc3sb = consts.tile([1, 40], F32, tag="bfc3sb")
        nc.sync.dma_start(bfc3sb[:, :], bfc3_d[:, :])

        h1sb = persist.tile([128, 2], F32, tag="h1sb")
        for mt in range(2):
            h1_ps = pss.tile([128, 1], F32, tag="a")
            for c in range(4):
                nc.tensor.matmul(
                    h1_ps[:, :], lhsT=wfc1sb[:, c, mt * 128:(mt + 1) * 128],
                    rhs=yTr[:, c:c + 1],
                    start=(c == 0), stop=(c == 3), skip_group_check=True)
            nc.scalar.activation(
                h1sb[:, mt:mt + 1], h1_ps[:, :], AF.Prelu,
                bias=bfc1sb[:, mt:mt + 1], scale=1.0, alpha=NEG_SLOPE)
        h2sb = persist.tile([128, 1], F32, tag="h2sb")
        h2_ps = pss.tile([128, 1], F32, tag="a")
        for c in range(2):
            nc.tensor.matmul(
                h2_ps[:, :], lhsT=wfc2sb[:, c, :], rhs=h1sb[:, c:c + 1],
                start=(c == 0), stop=(c == 1), skip_group_check=True)
        nc.scalar.activation(
            h2sb[:, :], h2_ps[:, :], AF.Prelu,
            bias=bfc2sb[:, :], scale=1.0, alpha=NEG_SLOPE)

        out_ps = pss.tile([40, 1], F32, tag="a")
        nc.tensor.matmul(
            out_ps[:, :], lhsT=wfc3sb[:, :], rhs=h2sb[:, :],
            start=True, stop=False, skip_group_check=True)
        nc.tensor.matmul(
            out_ps[:, :], lhsT=bfc3sb[:, :], rhs=onescol[0:1, :],
            start=False, stop=True, skip_group_check=True)
        out_sb = persist.tile([40, 1], F32, tag="out_sb")
        nc.scalar.activation(out_sb[:, :], out_ps[:, :], AF.Copy)
        nc.sync.dma_start(out_d[:, :], out_sb[:, :])


# ---------------------------------------------------------------------------
# harness entry point
# ---------------------------------------------------------------------------
_NC_CACHE = {}


def _get_nc():
    if 'nc' not in _NC_CACHE:
        _NC_CACHE['nc'] = build_nc()
    return _NC_CACHE['nc']


def kernel(**inputs):
    """Full-batch EdgeCNN forward. x: (8, 1024, 3) -> (8, 40) float32.

    Pure data parallel: batch element b runs on NeuronCore b.
    """
    from concourse.bass_utils import run_bass_kernel_spmd

    inp = {k: np.asarray(v) for k, v in inputs.items()}
    prep = host_prep(inp)
    nc = _get_nc()
    in_maps = []
    for b in range(8):
        m = {'x': np.ascontiguousarray(inp['x'][b]).astype(np.float32)}
        m.update(prep)
        in_maps.append(m)
    res = run_bass_kernel_spmd(nc, in_maps, core_ids=list(range(8)))
    out = np.stack([res.results[b]['out'].reshape(40) for b in range(8)])
    return out.astype(np.float32)


# revision 5
# speedup vs baseline: 1.1737x; 1.0024x over previous
"""EdgeCNN (DGCNN) Bass/Tile kernel for TRN2 — one batch element per core.

Per edge-conv layer (N=1024 points, K=20 neighbors):
  1. PE: packed-key matmul  pd[n,j] = 2<xn,xj> - S[j] - S[n]   (PSUM, fp32)
  2. DVE: one-pass (pd & ~0x3FF) | j  -> packed keys (scalar_tensor_tensor)
  3. DVE: 3x max8 + 2x match_replace -> top-20 packed keys; extract j
  4. idx -> DRAM -> read back in dma_gather wrapped layout (partition = n%16)
  5. SWDGE dma_gather (4 queues round-robin; one Q7 core-pair per queue) of
     rows of a = x @ (g~ Wn)^T; DVE strided reduce_max over k
  6. PE: transpose(m) + c-matmul (c = x @ (g~(Wc-Wn))^T + b) accumulated in PSUM
  7. ACT: leaky-relu (Prelu alpha=0.2) PSUM -> next layer xT
Head: conv5 via K-chunk accumulation, global max-pool, 3 FC layers on PE.
"""

import contextlib

import numpy as np

import concourse.bass as bass
import concourse.bacc as bacc
import concourse.mybir as mybir
from concourse.tile import TileContext
from concourse.masks import make_identity

F32 = mybir.dt.float32
U32 = mybir.dt.uint32
I16 = mybir.dt.int16
F16 = mybir.dt.float16
AF = mybir.ActivationFunctionType
ALU = mybir.AluOpType
AX = mybir.AxisListType

N = 1024
KNN = 20
NT = 8
NEG_SLOPE = 0.2
BNI = np.float32(1.0 / np.sqrt(1.0 + 1e-5))
LAYERS = [(3, 64), (64, 64), (64, 128), (128, 256)]
NEG_BIG = -3.0e38
NQ = 4  # SWDGE queues


def host_prep(inp):
    """Fold BN scale/bias into weights; transpose for device layout."""
    d = {}
    for li, (C, O) in enumerate(LAYERS, start=1):
        W = inp[f'W{li}'].astype(np.float32)
        g = inp[f'g{li}'].astype(np.float32)
        b = inp[f'b{li}'].astype(np.float32)
        gt = g * BNI
        Wn = W[:, :C]
        Wc = W[:, C:]
        d[f'wnt{li}'] = np.ascontiguousarray((gt[:, None] * Wn).T)          # (C, O)
        d[f'wdt{li}'] = np.ascontiguousarray((gt[:, None] * (Wc - Wn)).T)   # (C, O)
        d[f'bs{li}'] = b.reshape(1, O).copy()
    g5 = inp['g5'].astype(np.float32) * BNI
    d['w5t'] = np.ascontiguousarray((g5[:, None] * inp['W5']).T)            # (512, 512)
    d['b5'] = inp['b5'].reshape(1, 512).astype(np.float32).copy()
    g1 = inp['bng1'].astype(np.float32) * BNI
    d['wfc1'] = np.ascontiguousarray((g1[:, None] * inp['fc1_w']).T)        # (512, 256)
    bf1 = g1 * inp['fc1_b'].astype(np.float32) + inp['bnb1'].astype(np.float32)
    d['bfc1'] = np.ascontiguousarray(bf1.reshape(2, 128).T)                 # (128, 2)
    g2 = inp['bng2'].astype(np.float32) * BNI
    d['wfc2'] = np.ascontiguousarray((g2[:, None] * inp['fc2_w']).T)        # (256, 128)
    bf2 = g2 * inp['fc2_b'].astype(np.float32) + inp['bnb2'].astype(np.float32)
    d['bfc2'] = np.ascontiguousarray(bf2.reshape(128, 1))                   # (128, 1)
    d['wfc3'] = np.ascontiguousarray(inp['fc3_w'].T)                        # (128, 40)
    d['bfc3'] = inp['fc3_b'].reshape(1, 40).astype(np.float32).copy()
    return d


def build_nc(stage='full'):
    # The stock cost model assumes the plain-SWDGE descriptor rate
    # (0.34 ns/desc); dma_gather's per-idx generation measures ~7.5 ns. The
    # Tile scheduler needs the real number to hide gathers behind compute.
    import concourse.hw_specs as hw_specs
    hw_specs.TRN2Spec.SWDGE_NS_PER_DESCRIPTOR = 7.5
    nc = bacc.Bacc("TRN2", target_bir_lowering=False, debug=False, num_devices=8,
                   num_swdge_queues=NQ)
    with TileContext(nc) as tc:
        _trace(nc, tc, stage)
    nc.compile()
    return nc


def _trace(nc, tc, stage='full'):
    with contextlib.ExitStack() as ctx:
        dram = ctx.enter_context(tc.tile_pool(name="dram", bufs=1, space="DRAM"))
        consts = ctx.enter_context(tc.tile_pool(name="consts", bufs=1))
        persist = ctx.enter_context(tc.tile_pool(name="persist", bufs=1))
        sb = ctx.enter_context(tc.tile_pool(name="sb", bufs=2))
        keyp = ctx.enter_context(tc.tile_pool(name="keyp", bufs=2))
        gath = ctx.enter_context(tc.tile_pool(name="gath", bufs=3))
        mp = ctx.enter_context(tc.tile_pool(name="mp", bufs=1))
        psb = ctx.enter_context(tc.tile_pool(name="psb", bufs=3, space="PSUM"))
        pss = ctx.enter_context(tc.tile_pool(name="pss", bufs=2, space="PSUM"))

        # ---- DRAM I/O ----
        x_d = dram.tile([N, 3], F32, kind="ExternalInput", uniquify=False, name="x")
        win = {}
        for li, (C, O) in enumerate(LAYERS, start=1):
            win[f'wnt{li}'] = dram.tile([C, O], F32, kind="ExternalInput", uniquify=False, name=f"wnt{li}")
            win[f'wdt{li}'] = dram.tile([C, O], F32, kind="ExternalInput", uniquify=False, name=f"wdt{li}")
            win[f'bs{li}'] = dram.tile([1, O], F32, kind="ExternalInput", uniquify=False, name=f"bs{li}")
        w5t_d = dram.tile([512, 512], F32, kind="ExternalInput", uniquify=False, name="w5t")
        b5_d = dram.tile([1, 512], F32, kind="ExternalInput", uniquify=False, name="b5")
        wfc1_d = dram.tile([512, 256], F32, kind="ExternalInput", uniquify=False, name="wfc1")
        bfc1_d = dram.tile([128, 2], F32, kind="ExternalInput", uniquify=False, name="bfc1")
        wfc2_d = dram.tile([256, 128], F32, kind="ExternalInput", uniquify=False, name="wfc2")
        bfc2_d = dram.tile([128, 1], F32, kind="ExternalInput", uniquify=False, name="bfc2")
        wfc3_d = dram.tile([128, 40], F32, kind="ExternalInput", uniquify=False, name="wfc3")
        bfc3_d = dram.tile([1, 40], F32, kind="ExternalInput", uniquify=False, name="bfc3")
        out_d = dram.tile([40, 1], F32, kind="ExternalOutput", uniquify=False, name="out")
        dbg_d = None
        if stage != 'full':
            dbg_d = dram.tile([128, N], F32, kind="ExternalOutput", uniquify=False, name="dbg")

        a_ds = {li: dram.tile([N, O], F16 if li == 4 else F32, name=f"a_d{li}")
                for li, (C, O) in enumerate(LAYERS, start=1)}
        jw_ds = {li: dram.tile([N * KNN // 16, 128], I16, name=f"jw_d{li}")
                 for li in range(1, 5)}

        # ---- consts ----
        iotaJ = consts.tile([128, N], U32, tag="iotaJ")
        nc.gpsimd.iota(iotaJ[:, :], [[1, N]], base=0, channel_multiplier=0)
        ident = consts.tile([128, 128], F32, tag="ident")
        make_identity(nc, ident[:, :])
        onescol = consts.tile([128, 1], F32, tag="onescol")
        nc.vector.memset(onescol[:, :], 1.0)
        onesrow = consts.tile([1, N], F32, tag="onesrow")
        nc.vector.memset(onesrow[:, :], 1.0)
        negones = consts.tile([1, N], F32, tag="negones")
        nc.vector.memset(negones[:, :], -1.0)
        maskc = consts.tile([128, 1], U32, tag="maskc")
        nc.vector.memset(maskc[:, :], 0xFFFFFC00)

        # persistent feature tensors
        x0T = persist.tile([3, N], F32, tag="x0T")
        x1T = persist.tile([64, N], F32, tag="x1T")
        x2T = persist.tile([64, N], F32, tag="x2T")
        x3T = persist.tile([128, N], F32, tag="x3T")
        x4Ta = persist.tile([128, N], F32, tag="x4Ta")
        x4Tb = persist.tile([128, N], F32, tag="x4Tb")

        # load x transposed: x_d is (N, 3) row-major
        xap = x_d[:, :]
        nc.sync.dma_start(
            x0T[:, :], bass.AP(xap.tensor, xap.offset, [[1, 3], [3, N]]))
        if stage == 'xload':
            nc.sync.dma_start(dbg_d[0:3, :], x0T[:, :])
            nc.sync.dma_start(out_d[:, :], onescol[0:40, :])
            return

        def edge_layer(li, xT, C, O, out_parts, post_tiles=None):
            a_d = a_ds[li]
            jw_d = jw_ds[li]
            jwap = jw_d[:, :]
            wnt = sb.tile([C, O], F32, tag="wnt")
            wdt = sb.tile([C, O], F32, tag="wdt")
            bs = sb.tile([1, O], F32, tag="bs")
            nc.sync.dma_start(wnt[:, :], win[f'wnt{li}'][:, :])
            nc.sync.dma_start(wdt[:, :], win[f'wdt{li}'][:, :])
            nc.sync.dma_start(bs[:, :], win[f'bs{li}'][:, :])

            jf = sb.tile([128, NT * KNN], F32, tag="jf")
            xsq = sb.tile([C, N], F32, tag="xsq")
            nc.scalar.activation(xsq[:, :], xT[:, :], AF.Square)
            x2d = sb.tile([C, N], F32, tag="x2d")
            nc.scalar.activation(x2d[:, :], xT[:, :], AF.Copy, bias=0.0, scale=2.0)

            SO2 = mp.tile([2, N], F32, tag="SO2")
            OS2 = mp.tile([2, N], F32, tag="OS2")
            negS = mp.tile([1, N], F32, tag="negS")
            nc.vector.memset(OS2[0:1, :], -1.0)
            for h in range(2):
                cols = slice(h * 512, (h + 1) * 512)
                S_ps = pss.tile([1, 512], F32, tag="a")
                nc.tensor.matmul(
                    S_ps[:, :], lhsT=onescol[0:C, :], rhs=xsq[:, cols],
                    start=True, stop=True, skip_group_check=True)
                nc.scalar.activation(
                    SO2[0:1, cols], S_ps[:, :], AF.Copy, bias=0.0, scale=1.0)
                nc.scalar.activation(
                    negS[0:1, cols], S_ps[:, :], AF.Copy, bias=0.0, scale=-1.0)
            nc.sync.dma_start(SO2[1:2, :], onesrow[0:1, :])
            nc.sync.dma_start(OS2[1:2, :], negS[0:1, :])

            # a-rows to DRAM first so gathers can start as soon as idx ready
            for t in range(NT):
                a_ps = pss.tile([128, O], F32, tag="a")
                nc.tensor.matmul(
                    a_ps[:, :], lhsT=xT[:, t * 128:(t + 1) * 128], rhs=wnt[:, :],
                    start=True, stop=True, skip_group_check=True)
                a_sb = sb.tile([128, O], F16 if li == 4 else F32, tag="a_sb")
                nc.scalar.activation(a_sb[:, :], a_ps[:, :], AF.Copy)
                nc.sync.dma_start(a_d[t * 128:(t + 1) * 128, :], a_sb[:, :])

            m = mp.tile([128, NT, O], F32, tag="m")
            ntl = NT
            if stage.startswith('topk1_'):
                ntl = int(stage.split('_')[1])
            for t in range(ntl):
                if stage in ('keys', 'pack', 'max1', 'mr1', 'ext', 'topkt0') and t > 0:
                    break
                tcols = slice(t * 128, (t + 1) * 128)
                kp = psb.tile([128, N], F32, tag="big")
                for h in range(2):
                    cols = slice(h * 512, (h + 1) * 512)
                    nc.tensor.matmul(
                        kp[:, cols], lhsT=xT[:, tcols], rhs=x2d[:, cols],
                        start=True, stop=False, skip_group_check=True)
                    nc.tensor.matmul(
                        kp[:, cols], lhsT=SO2[:, tcols], rhs=OS2[:, cols],
                        start=False, stop=True, skip_group_check=True)
                if stage == 'keys' and t == 0:
                    kcp = sb.tile([128, N], F32, tag="kcp")
                    nc.scalar.activation(kcp[:, :], kp[:, :], AF.Copy)
                    nc.sync.dma_start(dbg_d[:, :], kcp[:, :])
                    nc.sync.dma_start(out_d[:, :], onescol[0:40, :])
                    return 'stop'
                kb = keyp.tile([128, N], U32, tag="keysP")
                nc.vector.scalar_tensor_tensor(
                    out=kb[:, :], in0=kp[:, :].bitcast(U32), scalar=maskc[:, 0:1],
                    in1=iotaJ[:, :], op0=ALU.bitwise_and, op1=ALU.bitwise_or)
                if stage == 'pack' and t == 0:
                    kcp = sb.tile([128, N], F32, tag="kcp")
                    nc.vector.tensor_copy(kcp[:, :].bitcast(U32), kb[:, :])
                    nc.sync.dma_start(dbg_d[:, :], kcp[:, :])
                    nc.sync.dma_start(out_d[:, :], onescol[0:40, :])
                    return 'stop'
                kbf = kb[:, :].bitcast(F32)
                v24 = sb.tile([128, 24], F32, tag="v24")
                nc.vector.max(v24[:, 0:8], kbf)
                if stage == 'max1' and t == 0:
                    nc.sync.dma_start(dbg_d[:, 0:8], v24[:, 0:8])
                    nc.sync.dma_start(out_d[:, :], onescol[0:40, :])
                    return 'stop'
                nc.vector.match_replace(kbf, v24[:, 0:8], kbf, NEG_BIG)
                if stage == 'mr1' and t == 0:
                    nc.sync.dma_start(dbg_d[:, :], kb[:, :].bitcast(F32))
                    nc.sync.dma_start(out_d[:, :], onescol[0:40, :])
                    return 'stop'
                nc.vector.max(v24[:, 8:16], kbf)
                nc.vector.match_replace(kbf, v24[:, 8:16], kbf, NEG_BIG)
                nc.vector.max(v24[:, 16:24], kbf)
                if stage == 'ext' and t == 0:
                    nc.sync.dma_start(dbg_d[:, 0:24], v24[:, :])
                    nc.sync.dma_start(out_d[:, :], onescol[0:40, :])
                    return 'stop'
                j20 = sb.tile([128, KNN], U32, tag="j20")
                nc.vector.tensor_scalar(
                    j20[:, :], v24[:, 0:KNN].bitcast(U32), 0x3FF, None,
                    op0=ALU.bitwise_and)
                # j as fp32 values, accumulated for PE transpose
                nc.vector.tensor_copy(jf[:, t * KNN:(t + 1) * KNN], j20[:, :])
                if stage == 'topkt0' and t == 0:
                    nc.sync.dma_start(dbg_d[:, 0:KNN], jf[:, 0:KNN])
                    nc.sync.dma_start(out_d[:, :], onescol[0:40, :])
                    return 'stop'

                # per-tile idx wrap + gather on queue t%NQ
                jT_ps = pss.tile([KNN, 128], F32, tag="a")
                nc.tensor.matmul(
                    jT_ps[:, :], lhsT=jf[:, t * KNN:(t + 1) * KNN],
                    rhs=ident[:, 0:128], is_transpose=True, start=True, stop=True,
                    skip_group_check=True)
                jTi = sb.tile([KNN, 128], I16, tag="jTi")
                nc.vector.tensor_copy(jTi[:, :], jT_ps[:, :])
                dst = bass.AP(jwap.tensor, jwap.offset + t * 160 * 128,
                              [[1024, KNN], [128, 8], [1, 16]])
                nc.sync.dma_start(
                    dst, jTi[:, :].rearrange("k (h s) -> k h s", s=16))
                src_ap = bass.AP(jwap.tensor, jwap.offset + t * 160 * 128,
                                 [[128, 160], [1, 128]])
                idq = keyp.tile([128, 10 * 16], I16, tag="idxq", bufs=4)
                nc.scalar.dma_start_transpose(idq[:, :], src_ap)
                for rr in range(1, 8):
                    nc.scalar.dma_start(
                        idq[16 * rr:16 * (rr + 1), :], idq[0:16, :])

                gdt = F16 if li == 4 else F32
                g = gath.tile([128, KNN, O], gdt, tag="g", bufs=3)
                nc.gpsimd.dma_gather(
                    out_ap=g[:, :, :], in_ap=a_d[:, :],
                    idxs_ap=idq[:, :],
                    num_idxs=KNN * 128, num_idxs_reg=KNN * 128, elem_size=O,
                    single_packet=False, queue_num=t % NQ)
                gap = g[:, :, :]
                red_in = bass.AP(
                    gap.tensor, gap.offset,
                    [gap.ap[0], [1, O], [O, KNN]])
                nc.vector.tensor_reduce(
                    out=m[:, t, :], in_=red_in, axis=AX.X, op=ALU.max)

            if stage.startswith('topk1'):
                nc.sync.dma_start(dbg_d[0:128, 0:NT * KNN], jf[:, :])
                nc.sync.dma_start(out_d[:, :], onescol[0:40, :])
                return 'stop'

            if post_tiles is not None:
                post_tiles()

            # transpose m + c matmul + lrelu -> out_parts
            for ot, (op_ap, orow) in enumerate(out_parts):
                px = psb.tile([orow, N], F32, tag="big")
                for t in range(NT):
                    nc.tensor.matmul(
                        px[:, t * 128:(t + 1) * 128],
                        lhsT=m[:, t, ot * 128:ot * 128 + orow],
                        rhs=ident[:, 0:128],
                        is_transpose=True, start=(t % 4 == 0), stop=False,
                        skip_group_check=True)
                for h in range(2):
                    cols = slice(h * 512, (h + 1) * 512)
                    nc.tensor.matmul(
                        px[:, cols],
                        lhsT=wdt[:, ot * 128:ot * 128 + orow],
                        rhs=xT[:, cols],
                        start=False, stop=False, skip_group_check=True)
                    nc.tensor.matmul(
                        px[:, cols],
                        lhsT=bs[0:1, ot * 128:ot * 128 + orow],
                        rhs=onesrow[0:1, cols],
                        start=False, stop=True, skip_group_check=True)
                nc.scalar.activation(op_ap, px[:, :], AF.Prelu, alpha=NEG_SLOPE)

        w5sb = {}
        for ci, (rows, k0) in enumerate([(64, 0), (64, 64), (128, 128),
                                         (128, 256), (128, 384)]):
            w5c = consts.tile([rows, 512], F32, tag=f"w5c{ci}")
            nc.sync.dma_start(w5c[:, :], w5t_d[k0:k0 + rows, :])
            w5sb[ci] = w5c
        b5sb = consts.tile([1, 512], F32, tag="b5sb")
        nc.sync.dma_start(b5sb[:, :], b5_d[:, :])
        zpart = persist.tile([128, NT, 512], F32, tag="zpart")

        def zpart_fill():
            for t in range(NT):
                tcols = slice(t * 128, (t + 1) * 128)
                zp_ps = pss.tile([128, 512], F32, tag="a")
                for ci, (xt, rows) in enumerate(
                        [(x1T, 64), (x2T, 64), (x3T, 128)]):
                    nc.tensor.matmul(
                        zp_ps[:, :], lhsT=xt[:, tcols], rhs=w5sb[ci][:, :],
                        start=(ci == 0), stop=(ci == 2), skip_group_check=True)
                nc.scalar.activation(zpart[:, t, :], zp_ps[:, :], AF.Copy)

        r = edge_layer(1, x0T, 3, 64, [(x1T[:, :], 64)])
        if r == 'stop':
            return
        if stage == 'gath1':
            nc.sync.dma_start(dbg_d[0:64, :], x1T[:, :])
            nc.sync.dma_start(out_d[:, :], onescol[0:40, :])
            return
        edge_layer(2, x1T, 64, 64, [(x2T[:, :], 64)])
        edge_layer(3, x2T, 64, 128, [(x3T[:, :], 128)])
        edge_layer(4, x3T, 128, 256, [(x4Ta[:, :], 128), (x4Tb[:, :], 128)],
                   post_tiles=zpart_fill)

        # ---- head: conv5 (x4 chunks; x1-x3 partials precomputed) + max pool ----
        zmax = persist.tile([128, 512], F32, tag="zmax")
        for t in range(NT):
            tcols = slice(t * 128, (t + 1) * 128)
            z_ps = pss.tile([128, 512], F32, tag="a")
            for ci, (xt, rows, k0) in enumerate(
                    [(x4Ta, 128, 256), (x4Tb, 128, 384)]):
                nc.tensor.matmul(
                    z_ps[:, :], lhsT=xt[:, tcols], rhs=w5sb[3 + ci][:, :],
                    start=(ci == 0), stop=False, skip_group_check=True)
            nc.tensor.matmul(
                z_ps[:, :], lhsT=onesrow[0:1, tcols],
                rhs=b5sb[:, :], start=False, stop=True, skip_group_check=True)
            zsb = sb.tile([128, 512], F32, tag="zsb")
            nc.vector.tensor_tensor(
                out=zsb[:, :], in0=zpart[:, t, :], in1=z_ps[:, :], op=ALU.add)
            if t == 0:
                nc.scalar.activation(zmax[:, :], zsb[:, :], AF.Copy)
            else:
                nc.vector.tensor_tensor(
                    out=zmax[:, :], in0=zmax[:, :], in1=zsb[:, :], op=ALU.max)
        # partition tree-max 128 -> 1... then we need yT [128, 4] instead:
        # transpose zmax chunks and reduce along free dim.
        yT = persist.tile([128, 4], F32, tag="yT")
        for cchunk in range(4):
            zt_ps = pss.tile([128, 128], F32, tag="a")
            nc.tensor.matmul(
                zt_ps[:, :], lhsT=zmax[:, cchunk * 128:(cchunk + 1) * 128],
                rhs=ident[:, 0:128], is_transpose=True, start=True, stop=True,
                skip_group_check=True)
            nc.vector.tensor_reduce(
                out=yT[:, cchunk:cchunk + 1], in_=zt_ps[:, :],
                axis=AX.X, op=ALU.max)
        # leaky relu on yT
        yTr = persist.tile([128, 4], F32, tag="yTr")
        nc.scalar.activation(yTr[:, :], yT[:, :], AF.Prelu, alpha=NEG_SLOPE)

        # ---- FC head ----
        wfc1sb = consts.tile([128, 4, 256], F32, tag="wfc1sb")
        for c in range(4):
            nc.sync.dma_start(wfc1sb[:, c, :], wfc1_d[c * 128:(c + 1) * 128, :])
        bfc1sb = consts.tile([128, 2], F32, tag="bfc1sb")
        nc.sync.dma_start(bfc1sb[:, :], bfc1_d[:, :])
        wfc2sb = consts.tile([128, 2, 128], F32, tag="wfc2sb")
        for c in range(2):
            nc.sync.dma_start(wfc2sb[:, c, :], wfc2_d[c * 128:(c + 1) * 128, :])
        bfc2sb = consts.tile([128, 1], F32, tag="bfc2sb")
        nc.sync.dma_start(bfc2sb[:, :], bfc2_d[:, :])
        wfc3sb = consts.tile([128, 40], F32, tag="wfc3sb")
        nc.sync.dma_start(wfc3sb[:, :], wfc3_d[:, :])
        bfc3sb = consts.tile([1, 40], F32, tag="bfc3sb")
        nc.sync.dma_start(bfc3sb[:, :], bfc3_d[:, :])

        h1sb = persist.tile([128, 2], F32, tag="h1sb")
        for mt in range(2):
            h1_ps = pss.tile([128, 1], F32, tag="a")
            for c in range(4):
                nc.tensor.matmul(
                    h1_ps[:, :], lhsT=wfc1sb[:, c, mt * 128:(mt + 1) * 128],
                    rhs=yTr[:, c:c + 1],
                    start=(c == 0), stop=(c == 3), skip_group_check=True)
            nc.scalar.activation(
                h1sb[:, mt:mt + 1], h1_ps[:, :], AF.Prelu,
                bias=bfc1sb[:, mt:mt + 1], scale=1.0, alpha=NEG_SLOPE)
        h2sb = persist.tile([128, 1], F32, tag="h2sb")
        h2_ps = pss.tile([128, 1], F32, tag="a")
        for c in range(2):
            nc.tensor.matmul(
                h2_ps[:, :], lhsT=wfc2sb[:, c, :], rhs=h1sb[:, c:c + 1],
                start=(c == 0), stop=(c == 1), skip_group_check=True)
        nc.scalar.activation(
            h2sb[:, :], h2_ps[:, :], AF.Prelu,
            bias=bfc2sb[:, :], scale=1.0, alpha=NEG_SLOPE)

        out_ps = pss.tile([40, 1], F32, tag="a")
        nc.tensor.matmul(
            out_ps[:, :], lhsT=wfc3sb[:, :], rhs=h2sb[:, :],
            start=True, stop=False, skip_group_check=True)
        nc.tensor.matmul(
            out_ps[:, :], lhsT=bfc3sb[:, :], rhs=onescol[0:1, :],
            start=False, stop=True, skip_group_check=True)
        out_sb = persist.tile([40, 1], F32, tag="out_sb")
        nc.scalar.activation(out_sb[:, :], out_ps[:, :], AF.Copy)
        nc.sync.dma_start(out_d[:, :], out_sb[:, :])


# ---------------------------------------------------------------------------
# harness entry point
# ---------------------------------------------------------------------------
_NC_CACHE = {}


def _get_nc():
    if 'nc' not in _NC_CACHE:
        _NC_CACHE['nc'] = build_nc()
    return _NC_CACHE['nc']


def kernel(**inputs):
    """Full-batch EdgeCNN forward. x: (8, 1024, 3) -> (8, 40) float32.

    Pure data parallel: batch element b runs on NeuronCore b.
    """
    from concourse.bass_utils import run_bass_kernel_spmd

    inp = {k: np.asarray(v) for k, v in inputs.items()}
    prep = host_prep(inp)
    nc = _get_nc()
    in_maps = []
    for b in range(8):
        m = {'x': np.ascontiguousarray(inp['x'][b]).astype(np.float32)}
        m.update(prep)
        in_maps.append(m)
    res = run_bass_kernel_spmd(nc, in_maps, core_ids=list(range(8)))
    out = np.stack([res.results[b]['out'].reshape(40) for b in range(8)])
    return out.astype(np.float32)


# revision 7
# speedup vs baseline: 1.2054x; 1.0270x over previous
"""EdgeCNN (DGCNN) Bass/Tile kernel for TRN2 — one batch element per core.

Per edge-conv layer (N=1024 points, K=20 neighbors):
  1. PE: packed-key matmul  pd[n,j] = 2<xn,xj> - S[j] - S[n]   (PSUM, fp32)
  2. DVE: one-pass (pd & ~0x3FF) | j  -> packed keys (scalar_tensor_tensor)
  3. DVE: 3x max8 + 2x match_replace -> top-20 packed keys; extract j
  4. idx -> DRAM -> read back in dma_gather wrapped layout (partition = n%16)
  5. SWDGE dma_gather (4 queues round-robin; one Q7 core-pair per queue) of
     rows of a = x @ (g~ Wn)^T; DVE strided reduce_max over k
  6. PE: transpose(m) + c-matmul (c = x @ (g~(Wc-Wn))^T + b) accumulated in PSUM
  7. ACT: leaky-relu (Prelu alpha=0.2) PSUM -> next layer xT
Head: conv5 via K-chunk accumulation, global max-pool, 3 FC layers on PE.
"""

import contextlib

import numpy as np

import concourse.bass as bass
import concourse.bacc as bacc
import concourse.mybir as mybir
from concourse.tile import TileContext
from concourse.masks import make_identity

F32 = mybir.dt.float32
U32 = mybir.dt.uint32
I16 = mybir.dt.int16
F16 = mybir.dt.float16
AF = mybir.ActivationFunctionType
ALU = mybir.AluOpType
AX = mybir.AxisListType

N = 1024
KNN = 20
NT = 8
NEG_SLOPE = 0.2
BNI = np.float32(1.0 / np.sqrt(1.0 + 1e-5))
LAYERS = [(3, 64), (64, 64), (64, 128), (128, 256)]
NEG_BIG = -3.0e38
NQ = 4  # SWDGE queues


def host_prep(inp):
    """Fold BN scale/bias into weights; transpose for device layout."""
    d = {}
    for li, (C, O) in enumerate(LAYERS, start=1):
        W = inp[f'W{li}'].astype(np.float32)
        g = inp[f'g{li}'].astype(np.float32)
        b = inp[f'b{li}'].astype(np.float32)
        gt = g * BNI
        Wn = W[:, :C]
        Wc = W[:, C:]
        d[f'wnt{li}'] = np.ascontiguousarray((gt[:, None] * Wn).T)          # (C, O)
        d[f'wdt{li}'] = np.ascontiguousarray((gt[:, None] * (Wc - Wn)).T)   # (C, O)
        d[f'bs{li}'] = b.reshape(1, O).copy()
    g5 = inp['g5'].astype(np.float32) * BNI
    d['w5t'] = np.ascontiguousarray((g5[:, None] * inp['W5']).T)            # (512, 512)
    d['b5'] = inp['b5'].reshape(1, 512).astype(np.float32).copy()
    g1 = inp['bng1'].astype(np.float32) * BNI
    d['wfc1'] = np.ascontiguousarray((g1[:, None] * inp['fc1_w']).T)        # (512, 256)
    bf1 = g1 * inp['fc1_b'].astype(np.float32) + inp['bnb1'].astype(np.float32)
    d['bfc1'] = np.ascontiguousarray(bf1.reshape(2, 128).T)                 # (128, 2)
    g2 = inp['bng2'].astype(np.float32) * BNI
    d['wfc2'] = np.ascontiguousarray((g2[:, None] * inp['fc2_w']).T)        # (256, 128)
    bf2 = g2 * inp['fc2_b'].astype(np.float32) + inp['bnb2'].astype(np.float32)
    d['bfc2'] = np.ascontiguousarray(bf2.reshape(128, 1))                   # (128, 1)
    d['wfc3'] = np.ascontiguousarray(inp['fc3_w'].T)                        # (128, 40)
    d['bfc3'] = inp['fc3_b'].reshape(1, 40).astype(np.float32).copy()
    return d


def build_nc(stage='full'):
    # The stock cost model assumes the plain-SWDGE descriptor rate
    # (0.34 ns/desc); dma_gather's per-idx generation measures ~7.5 ns. The
    # Tile scheduler needs the real number to hide gathers behind compute.
    import concourse.hw_specs as hw_specs
    hw_specs.TRN2Spec.SWDGE_NS_PER_DESCRIPTOR = 7.5
    nc = bacc.Bacc("TRN2", target_bir_lowering=False, debug=False, num_devices=8,
                   num_swdge_queues=NQ)
    with TileContext(nc) as tc:
        _trace(nc, tc, stage)
    nc.compile()
    return nc


def _trace(nc, tc, stage='full'):
    with contextlib.ExitStack() as ctx:
        dram = ctx.enter_context(tc.tile_pool(name="dram", bufs=1, space="DRAM"))
        consts = ctx.enter_context(tc.tile_pool(name="consts", bufs=1))
        persist = ctx.enter_context(tc.tile_pool(name="persist", bufs=1))
        sb = ctx.enter_context(tc.tile_pool(name="sb", bufs=2))
        keyp = ctx.enter_context(tc.tile_pool(name="keyp", bufs=2))
        gath = ctx.enter_context(tc.tile_pool(name="gath", bufs=3))
        mp = ctx.enter_context(tc.tile_pool(name="mp", bufs=1))
        psb = ctx.enter_context(tc.tile_pool(name="psb", bufs=3, space="PSUM"))
        pss = ctx.enter_context(tc.tile_pool(name="pss", bufs=2, space="PSUM"))

        # ---- DRAM I/O ----
        x_d = dram.tile([N, 3], F32, kind="ExternalInput", uniquify=False, name="x")
        win = {}
        for li, (C, O) in enumerate(LAYERS, start=1):
            win[f'wnt{li}'] = dram.tile([C, O], F32, kind="ExternalInput", uniquify=False, name=f"wnt{li}")
            win[f'wdt{li}'] = dram.tile([C, O], F32, kind="ExternalInput", uniquify=False, name=f"wdt{li}")
            win[f'bs{li}'] = dram.tile([1, O], F32, kind="ExternalInput", uniquify=False, name=f"bs{li}")
        w5t_d = dram.tile([512, 512], F32, kind="ExternalInput", uniquify=False, name="w5t")
        b5_d = dram.tile([1, 512], F32, kind="ExternalInput", uniquify=False, name="b5")
        wfc1_d = dram.tile([512, 256], F32, kind="ExternalInput", uniquify=False, name="wfc1")
        bfc1_d = dram.tile([128, 2], F32, kind="ExternalInput", uniquify=False, name="bfc1")
        wfc2_d = dram.tile([256, 128], F32, kind="ExternalInput", uniquify=False, name="wfc2")
        bfc2_d = dram.tile([128, 1], F32, kind="ExternalInput", uniquify=False, name="bfc2")
        wfc3_d = dram.tile([128, 40], F32, kind="ExternalInput", uniquify=False, name="wfc3")
        bfc3_d = dram.tile([1, 40], F32, kind="ExternalInput", uniquify=False, name="bfc3")
        out_d = dram.tile([40, 1], F32, kind="ExternalOutput", uniquify=False, name="out")
        dbg_d = None
        if stage != 'full':
            dbg_d = dram.tile([128, N], F32, kind="ExternalOutput", uniquify=False, name="dbg")

        a_ds = {li: dram.tile([N, O], F16 if li == 4 else F32, name=f"a_d{li}")
                for li, (C, O) in enumerate(LAYERS, start=1)}
        jw_ds = {li: dram.tile([N * KNN // 16, 128], I16, name=f"jw_d{li}")
                 for li in range(1, 5)}

        # ---- consts ----
        iotaJ = consts.tile([128, N], U32, tag="iotaJ")
        nc.gpsimd.iota(iotaJ[:, :], [[1, N]], base=0, channel_multiplier=0)
        ident = consts.tile([128, 128], F32, tag="ident")
        make_identity(nc, ident[:, :])
        onescol = consts.tile([128, 1], F32, tag="onescol")
        nc.vector.memset(onescol[:, :], 1.0)
        onesrow = consts.tile([1, N], F32, tag="onesrow")
        nc.vector.memset(onesrow[:, :], 1.0)
        negones = consts.tile([1, N], F32, tag="negones")
        nc.vector.memset(negones[:, :], -1.0)
        maskc = consts.tile([128, 1], U32, tag="maskc")
        nc.vector.memset(maskc[:, :], 0xFFFFFC00)

        # persistent feature tensors
        x0T = persist.tile([3, N], F32, tag="x0T")
        x1T = persist.tile([64, N], F32, tag="x1T")
        x2T = persist.tile([64, N], F32, tag="x2T")
        x3T = persist.tile([128, N], F32, tag="x3T")
        x4Ta = persist.tile([128, N], F32, tag="x4Ta")
        x4Tb = persist.tile([128, N], F32, tag="x4Tb")

        # load x transposed: x_d is (N, 3) row-major
        xap = x_d[:, :]
        nc.sync.dma_start(
            x0T[:, :], bass.AP(xap.tensor, xap.offset, [[1, 3], [3, N]]))
        if stage == 'xload':
            nc.sync.dma_start(dbg_d[0:3, :], x0T[:, :])
            nc.sync.dma_start(out_d[:, :], onescol[0:40, :])
            return

        def edge_layer(li, xT, C, O, out_parts, post_tiles=None):
            a_d = a_ds[li]
            jw_d = jw_ds[li]
            jwap = jw_d[:, :]
            wnt = sb.tile([C, O], F32, tag="wnt")
            wdt = sb.tile([C, O], F32, tag="wdt")
            bs = sb.tile([1, O], F32, tag="bs")
            nc.sync.dma_start(wnt[:, :], win[f'wnt{li}'][:, :])
            nc.sync.dma_start(wdt[:, :], win[f'wdt{li}'][:, :])
            nc.sync.dma_start(bs[:, :], win[f'bs{li}'][:, :])

            jf = sb.tile([128, NT * KNN], F32, tag="jf")
            xsq = sb.tile([C, N], F32, tag="xsq")
            nc.scalar.activation(xsq[:, :], xT[:, :], AF.Square)
            x2d = sb.tile([C, N], F32, tag="x2d")
            nc.scalar.activation(x2d[:, :], xT[:, :], AF.Copy, bias=0.0, scale=2.0)

            SO2 = mp.tile([2, N], F32, tag="SO2")
            OS2 = mp.tile([2, N], F32, tag="OS2")
            negS = mp.tile([1, N], F32, tag="negS")
            nc.vector.memset(OS2[0:1, :], -1.0)
            for h in range(2):
                cols = slice(h * 512, (h + 1) * 512)
                S_ps = pss.tile([1, 512], F32, tag="a")
                nc.tensor.matmul(
                    S_ps[:, :], lhsT=onescol[0:C, :], rhs=xsq[:, cols],
                    start=True, stop=True, skip_group_check=True)
                nc.scalar.activation(
                    SO2[0:1, cols], S_ps[:, :], AF.Copy, bias=0.0, scale=1.0)
                nc.scalar.activation(
                    negS[0:1, cols], S_ps[:, :], AF.Copy, bias=0.0, scale=-1.0)
            nc.sync.dma_start(SO2[1:2, :], onesrow[0:1, :])
            nc.sync.dma_start(OS2[1:2, :], negS[0:1, :])

            # a-rows to DRAM first so gathers can start as soon as idx ready
            for t in range(NT):
                a_ps = pss.tile([128, O], F32, tag="a")
                nc.tensor.matmul(
                    a_ps[:, :], lhsT=xT[:, t * 128:(t + 1) * 128], rhs=wnt[:, :],
                    start=True, stop=True, skip_group_check=True)
                a_sb = sb.tile([128, O], F16 if li == 4 else F32, tag="a_sb")
                nc.scalar.activation(a_sb[:, :], a_ps[:, :], AF.Copy)
                nc.sync.dma_start(a_d[t * 128:(t + 1) * 128, :], a_sb[:, :])

            m = mp.tile([128, NT, O], F32, tag="m")
            pending_g = []
            ntl = NT
            if stage.startswith('topk1_'):
                ntl = int(stage.split('_')[1])
            for t in range(ntl):
                if stage in ('keys', 'pack', 'max1', 'mr1', 'ext', 'topkt0') and t > 0:
                    break
                tcols = slice(t * 128, (t + 1) * 128)
                kp = psb.tile([128, N], F32, tag="big")
                for h in range(2):
                    cols = slice(h * 512, (h + 1) * 512)
                    nc.tensor.matmul(
                        kp[:, cols], lhsT=xT[:, tcols], rhs=x2d[:, cols],
                        start=True, stop=False, skip_group_check=True)
                    nc.tensor.matmul(
                        kp[:, cols], lhsT=SO2[:, tcols], rhs=OS2[:, cols],
                        start=False, stop=True, skip_group_check=True)
                if stage == 'keys' and t == 0:
                    kcp = sb.tile([128, N], F32, tag="kcp")
                    nc.scalar.activation(kcp[:, :], kp[:, :], AF.Copy)
                    nc.sync.dma_start(dbg_d[:, :], kcp[:, :])
                    nc.sync.dma_start(out_d[:, :], onescol[0:40, :])
                    return 'stop'
                kb = keyp.tile([128, N], U32, tag="keysP")
                nc.vector.scalar_tensor_tensor(
                    out=kb[:, :], in0=kp[:, :].bitcast(U32), scalar=maskc[:, 0:1],
                    in1=iotaJ[:, :], op0=ALU.bitwise_and, op1=ALU.bitwise_or)
                if stage == 'pack' and t == 0:
                    kcp = sb.tile([128, N], F32, tag="kcp")
                    nc.vector.tensor_copy(kcp[:, :].bitcast(U32), kb[:, :])
                    nc.sync.dma_start(dbg_d[:, :], kcp[:, :])
                    nc.sync.dma_start(out_d[:, :], onescol[0:40, :])
                    return 'stop'
                kbf = kb[:, :].bitcast(F32)
                v24 = sb.tile([128, 24], F32, tag="v24")
                nc.vector.max(v24[:, 0:8], kbf)
                if stage == 'max1' and t == 0:
                    nc.sync.dma_start(dbg_d[:, 0:8], v24[:, 0:8])
                    nc.sync.dma_start(out_d[:, :], onescol[0:40, :])
                    return 'stop'
                nc.vector.match_replace(kbf, v24[:, 0:8], kbf, NEG_BIG)
                if stage == 'mr1' and t == 0:
                    nc.sync.dma_start(dbg_d[:, :], kb[:, :].bitcast(F32))
                    nc.sync.dma_start(out_d[:, :], onescol[0:40, :])
                    return 'stop'
                nc.vector.max(v24[:, 8:16], kbf)
                nc.vector.match_replace(kbf, v24[:, 8:16], kbf, NEG_BIG)
                nc.vector.max(v24[:, 16:24], kbf)
                if stage == 'ext' and t == 0:
                    nc.sync.dma_start(dbg_d[:, 0:24], v24[:, :])
                    nc.sync.dma_start(out_d[:, :], onescol[0:40, :])
                    return 'stop'
                j20 = sb.tile([128, KNN], U32, tag="j20")
                nc.vector.tensor_scalar(
                    j20[:, :], v24[:, 0:KNN].bitcast(U32), 0x3FF, None,
                    op0=ALU.bitwise_and)
                # j as fp32 values, accumulated for PE transpose
                nc.vector.tensor_copy(jf[:, t * KNN:(t + 1) * KNN], j20[:, :])
                if stage == 'topkt0' and t == 0:
                    nc.sync.dma_start(dbg_d[:, 0:KNN], jf[:, 0:KNN])
                    nc.sync.dma_start(out_d[:, :], onescol[0:40, :])
                    return 'stop'

                # per-tile idx wrap + gather on queue t%NQ
                jT_ps = pss.tile([KNN, 128], F32, tag="a")
                nc.tensor.matmul(
                    jT_ps[:, :], lhsT=jf[:, t * KNN:(t + 1) * KNN],
                    rhs=ident[:, 0:128], is_transpose=True, start=True, stop=True,
                    skip_group_check=True)
                jTi = sb.tile([KNN, 128], I16, tag="jTi")
                nc.vector.tensor_copy(jTi[:, :], jT_ps[:, :])
                dst = bass.AP(jwap.tensor, jwap.offset + t * 160 * 128,
                              [[1024, KNN], [128, 8], [1, 16]])
                nc.sync.dma_start(
                    dst, jTi[:, :].rearrange("k (h s) -> k h s", s=16))
                src_ap = bass.AP(jwap.tensor, jwap.offset + t * 160 * 128,
                                 [[128, 160], [1, 128]])
                idq = keyp.tile([128, 10 * 16], I16, tag="idxq", bufs=4)
                nc.scalar.dma_start_transpose(idq[:, :], src_ap)
                for rr in range(1, 8):
                    nc.scalar.dma_start(
                        idq[16 * rr:16 * (rr + 1), :], idq[0:16, :])

                gdt = F16 if li == 4 else F32
                g = gath.tile([128, KNN, O], gdt, tag="g", bufs=4)
                nc.gpsimd.dma_gather(
                    out_ap=g[:, :, :], in_ap=a_d[:, :],
                    idxs_ap=idq[:, :],
                    num_idxs=KNN * 128, num_idxs_reg=KNN * 128, elem_size=O,
                    single_packet=False, queue_num=t % NQ)
                pending_g.append((t, g))
                # lag the reduce 2 tiles so the DVE stream (strict program
                # order) never head-of-line blocks on an in-flight gather
                if len(pending_g) > 2:
                    tp, gp = pending_g.pop(0)
                    gap = gp[:, :, :]
                    red_in = bass.AP(
                        gap.tensor, gap.offset,
                        [gap.ap[0], [1, O], [O, KNN]])
                    nc.vector.tensor_reduce(
                        out=m[:, tp, :], in_=red_in, axis=AX.X, op=ALU.max)

            for tp, gp in pending_g:
                gap = gp[:, :, :]
                red_in = bass.AP(
                    gap.tensor, gap.offset,
                    [gap.ap[0], [1, O], [O, KNN]])
                nc.vector.tensor_reduce(
                    out=m[:, tp, :], in_=red_in, axis=AX.X, op=ALU.max)

            if stage.startswith('topk1'):
                nc.sync.dma_start(dbg_d[0:128, 0:NT * KNN], jf[:, :])
                nc.sync.dma_start(out_d[:, :], onescol[0:40, :])
                return 'stop'

            if post_tiles is not None:
                post_tiles()

            # transpose m + c matmul + lrelu -> out_parts
            for ot, (op_ap, orow) in enumerate(out_parts):
                px = psb.tile([orow, N], F32, tag="big")
                for t in range(NT):
                    nc.tensor.matmul(
                        px[:, t * 128:(t + 1) * 128],
                        lhsT=m[:, t, ot * 128:ot * 128 + orow],
                        rhs=ident[:, 0:128],
                        is_transpose=True, start=(t % 4 == 0), stop=False,
                        skip_group_check=True)
                for h in range(2):
                    cols = slice(h * 512, (h + 1) * 512)
                    nc.tensor.matmul(
                        px[:, cols],
                        lhsT=wdt[:, ot * 128:ot * 128 + orow],
                        rhs=xT[:, cols],
                        start=False, stop=False, skip_group_check=True)
                    nc.tensor.matmul(
                        px[:, cols],
                        lhsT=bs[0:1, ot * 128:ot * 128 + orow],
                        rhs=onesrow[0:1, cols],
                        start=False, stop=True, skip_group_check=True)
                nc.scalar.activation(op_ap, px[:, :], AF.Prelu, alpha=NEG_SLOPE)

        w5sb = {}
        for ci, (rows, k0) in enumerate([(64, 0), (64, 64), (128, 128),
                                         (128, 256), (128, 384)]):
            w5c = consts.tile([rows, 512], F32, tag=f"w5c{ci}")
            nc.sync.dma_start(w5c[:, :], w5t_d[k0:k0 + rows, :])
            w5sb[ci] = w5c
        b5sb = consts.tile([1, 512], F32, tag="b5sb")
        nc.sync.dma_start(b5sb[:, :], b5_d[:, :])
        zpart = persist.tile([128, NT, 512], F32, tag="zpart")

        def zpart_fill():
            for t in range(NT):
                tcols = slice(t * 128, (t + 1) * 128)
                zp_ps = pss.tile([128, 512], F32, tag="a")
                for ci, (xt, rows) in enumerate(
                        [(x1T, 64), (x2T, 64), (x3T, 128)]):
                    nc.tensor.matmul(
                        zp_ps[:, :], lhsT=xt[:, tcols], rhs=w5sb[ci][:, :],
                        start=(ci == 0), stop=(ci == 2), skip_group_check=True)
                nc.scalar.activation(zpart[:, t, :], zp_ps[:, :], AF.Copy)

        r = edge_layer(1, x0T, 3, 64, [(x1T[:, :], 64)])
        if r == 'stop':
            return
        if stage == 'gath1':
            nc.sync.dma_start(dbg_d[0:64, :], x1T[:, :])
            nc.sync.dma_start(out_d[:, :], onescol[0:40, :])
            return
        edge_layer(2, x1T, 64, 64, [(x2T[:, :], 64)])
        edge_layer(3, x2T, 64, 128, [(x3T[:, :], 128)])
        edge_layer(4, x3T, 128, 256, [(x4Ta[:, :], 128), (x4Tb[:, :], 128)],
                   post_tiles=zpart_fill)

        # ---- head: conv5 (x4 chunks; x1-x3 partials precomputed) + max pool ----
        zmax = persist.tile([128, 512], F32, tag="zmax")
        for t in range(NT):
            tcols = slice(t * 128, (t + 1) * 128)
            z_ps = pss.tile([128, 512], F32, tag="a")
            for ci, (xt, rows, k0) in enumerate(
                    [(x4Ta, 128, 256), (x4Tb, 128, 384)]):
                nc.tensor.matmul(
                    z_ps[:, :], lhsT=xt[:, tcols], rhs=w5sb[3 + ci][:, :],
                    start=(ci == 0), stop=False, skip_group_check=True)
            nc.tensor.matmul(
                z_ps[:, :], lhsT=onesrow[0:1, tcols],
                rhs=b5sb[:, :], start=False, stop=True, skip_group_check=True)
            zsb = sb.tile([128, 512], F32, tag="zsb")
            nc.vector.tensor_tensor(
                out=zsb[:, :], in0=zpart[:, t, :], in1=z_ps[:, :], op=ALU.add)
            if t == 0:
                nc.scalar.activation(zmax[:, :], zsb[:, :], AF.Copy)
            else:
                nc.vector.tensor_tensor(
                    out=zmax[:, :], in0=zmax[:, :], in1=zsb[:, :], op=ALU.max)
        # partition tree-max 128 -> 1... then we need yT [128, 4] instead:
        # transpose zmax chunks and reduce along free dim.
        yT = persist.tile([128, 4], F32, tag="yT")
        for cchunk in range(4):
            zt_ps = pss.tile([128, 128], F32, tag="a")
            nc.tensor.matmul(
                zt_ps[:, :], lhsT=zmax[:, cchunk * 128:(cchunk + 1) * 128],
                rhs=ident[:, 0:128], is_transpose=True, start=True, stop=True,
                skip_group_check=True)
            nc.vector.tensor_reduce(
                out=yT[:, cchunk:cchunk + 1], in_=zt_ps[:, :],
                axis=AX.X, op=ALU.max)
        # leaky relu on yT
        yTr = persist.tile([128, 4], F32, tag="yTr")
        nc.scalar.activation(yTr[:, :], yT[:, :], AF.Prelu, alpha=NEG_SLOPE)

        # ---- FC head ----
        wfc1sb = consts.tile([128, 4, 256], F32, tag="wfc1sb")
        for c in range(4):
            nc.sync.dma_start(wfc1sb[:, c, :], wfc1_d[c * 128:(c + 1) * 128, :])
        bfc1sb = consts.tile([128, 2], F32, tag="bfc1sb")
        nc.sync.dma_start(bfc1sb[:, :], bfc1_d[:, :])
        wfc2sb = consts.tile([128, 2, 128], F32, tag="wfc2sb")
        for c in range(2):
            nc.sync.dma_start(wfc2sb[:, c, :], wfc2_d[c * 128:(c + 1) * 128, :])
        bfc2sb = consts.tile([128, 1], F32, tag="bfc2sb")
        nc.sync.dma_start(bfc2sb[:, :], bfc2_d[:, :])
        wfc3sb = consts.tile([128, 40], F32, tag="wfc3sb")
        nc.sync.dma_start(wfc3sb[:, :], wfc3_d[:, :])
        bfc3sb = consts.tile([1, 40], F32, tag="bfc3sb")
        nc.sync.dma_start(bfc3sb[:, :], bfc3_d[:, :])

        h1sb = persist.tile([128, 2], F32, tag="h1sb")
        for mt in range(2):
            h1_ps = pss.tile([128, 1], F32, tag="a")
            for c in range(4):
                nc.tensor.matmul(
                    h1_ps[:, :], lhsT=wfc1sb[:, c, mt * 128:(mt + 1) * 128],
                    rhs=yTr[:, c:c + 1],
                    start=(c == 0), stop=(c == 3), skip_group_check=True)
            nc.scalar.activation(
                h1sb[:, mt:mt + 1], h1_ps[:, :], AF.Prelu,
                bias=bfc1sb[:, mt:mt + 1], scale=1.0, alpha=NEG_SLOPE)
        h2sb = persist.tile([128, 1], F32, tag="h2sb")
        h2_ps = pss.tile([128, 1], F32, tag="a")
        for c in range(2):
            nc.tensor.matmul(
                h2_ps[:, :], lhsT=wfc2sb[:, c, :], rhs=h1sb[:, c:c + 1],
                start=(c == 0), stop=(c == 1), skip_group_check=True)
        nc.scalar.activation(
            h2sb[:, :], h2_ps[:, :], AF.Prelu,
            bias=bfc2sb[:, :], scale=1.0, alpha=NEG_SLOPE)

        out_ps = pss.tile([40, 1], F32, tag="a")
        nc.tensor.matmul(
            out_ps[:, :], lhsT=wfc3sb[:, :], rhs=h2sb[:, :],
            start=True, stop=False, skip_group_check=True)
        nc.tensor.matmul(
            out_ps[:, :], lhsT=bfc3sb[:, :], rhs=onescol[0:1, :],
            start=False, stop=True, skip_group_check=True)
        out_sb = persist.tile([40, 1], F32, tag="out_sb")
        nc.scalar.activation(out_sb[:, :], out_ps[:, :], AF.Copy)
        nc.sync.dma_start(out_d[:, :], out_sb[:, :])


# ---------------------------------------------------------------------------
# harness entry point
# ---------------------------------------------------------------------------
_NC_CACHE = {}


def _get_nc():
    if 'nc' not in _NC_CACHE:
        _NC_CACHE['nc'] = build_nc()
    return _NC_CACHE['nc']


def kernel(**inputs):
    """Full-batch EdgeCNN forward. x: (8, 1024, 3) -> (8, 40) float32.

    Pure data parallel: batch element b runs on NeuronCore b.
    """
    from concourse.bass_utils import run_bass_kernel_spmd

    inp = {k: np.asarray(v) for k, v in inputs.items()}
    prep = host_prep(inp)
    nc = _get_nc()
    in_maps = []
    for b in range(8):
        m = {'x': np.ascontiguousarray(inp['x'][b]).astype(np.float32)}
        m.update(prep)
        in_maps.append(m)
    res = run_bass_kernel_spmd(nc, in_maps, core_ids=list(range(8)))
    out = np.stack([res.results[b]['out'].reshape(40) for b in range(8)])
    return out.astype(np.float32)


# revision 10
# speedup vs baseline: 1.2807x; 1.0625x over previous
"""EdgeCNN (DGCNN) Bass/Tile kernel for TRN2 — one batch element per core.

Per edge-conv layer (N=1024 points, K=20 neighbors):
  1. PE: packed-key matmul  pd[n,j] = 2<xn,xj> - S[j] - S[n]   (PSUM, fp32)
  2. DVE: one-pass (pd & ~0x3FF) | j  -> packed keys (scalar_tensor_tensor)
  3. DVE: 3x max8 + 2x match_replace -> top-20 packed keys; extract j
  4. idx -> DRAM -> read back in dma_gather wrapped layout (partition = n%16)
  5. SWDGE dma_gather (4 queues round-robin; one Q7 core-pair per queue) of
     rows of a = x @ (g~ Wn)^T; DVE strided reduce_max over k
  6. PE: transpose(m) + c-matmul (c = x @ (g~(Wc-Wn))^T + b) accumulated in PSUM
  7. ACT: leaky-relu (Prelu alpha=0.2) PSUM -> next layer xT
Head: conv5 via K-chunk accumulation, global max-pool, 3 FC layers on PE.
"""

import contextlib

import numpy as np

import concourse.bass as bass
import concourse.bacc as bacc
import concourse.mybir as mybir
from concourse.tile import TileContext
from concourse.masks import make_identity

F32 = mybir.dt.float32
U32 = mybir.dt.uint32
I16 = mybir.dt.int16
F16 = mybir.dt.float16
AF = mybir.ActivationFunctionType
ALU = mybir.AluOpType
AX = mybir.AxisListType

N = 1024
KNN = 20
NT = 8
NEG_SLOPE = 0.2
BNI = np.float32(1.0 / np.sqrt(1.0 + 1e-5))
LAYERS = [(3, 64), (64, 64), (64, 128), (128, 256)]
NEG_BIG = -3.0e38
NQ = 4  # SWDGE queues


def host_prep(inp):
    """Fold BN scale/bias into weights; transpose for device layout."""
    d = {}
    for li, (C, O) in enumerate(LAYERS, start=1):
        W = inp[f'W{li}'].astype(np.float32)
        g = inp[f'g{li}'].astype(np.float32)
        b = inp[f'b{li}'].astype(np.float32)
        gt = g * BNI
        Wn = W[:, :C]
        Wc = W[:, C:]
        d[f'wnt{li}'] = np.ascontiguousarray((gt[:, None] * Wn).T)          # (C, O)
        d[f'wdt{li}'] = np.ascontiguousarray((gt[:, None] * (Wc - Wn)).T)   # (C, O)
        d[f'bs{li}'] = b.reshape(1, O).copy()
    g5 = inp['g5'].astype(np.float32) * BNI
    d['w5t'] = np.ascontiguousarray((g5[:, None] * inp['W5']).T)            # (512, 512)
    d['b5'] = inp['b5'].reshape(1, 512).astype(np.float32).copy()
    g1 = inp['bng1'].astype(np.float32) * BNI
    d['wfc1'] = np.ascontiguousarray((g1[:, None] * inp['fc1_w']).T)        # (512, 256)
    bf1 = g1 * inp['fc1_b'].astype(np.float32) + inp['bnb1'].astype(np.float32)
    d['bfc1'] = np.ascontiguousarray(bf1.reshape(2, 128).T)                 # (128, 2)
    g2 = inp['bng2'].astype(np.float32) * BNI
    d['wfc2'] = np.ascontiguousarray((g2[:, None] * inp['fc2_w']).T)        # (256, 128)
    bf2 = g2 * inp['fc2_b'].astype(np.float32) + inp['bnb2'].astype(np.float32)
    d['bfc2'] = np.ascontiguousarray(bf2.reshape(128, 1))                   # (128, 1)
    d['wfc3'] = np.ascontiguousarray(inp['fc3_w'].T)                        # (128, 40)
    d['bfc3'] = inp['fc3_b'].reshape(1, 40).astype(np.float32).copy()
    return d


def build_nc(stage='full'):
    # The stock cost model assumes the plain-SWDGE descriptor rate
    # (0.34 ns/desc); dma_gather's per-idx generation measures ~7.5 ns. The
    # Tile scheduler needs the real number to hide gathers behind compute.
    import concourse.hw_specs as hw_specs
    hw_specs.TRN2Spec.SWDGE_NS_PER_DESCRIPTOR = 7.5
    nc = bacc.Bacc("TRN2", target_bir_lowering=False, debug=False, num_devices=8,
                   num_swdge_queues=NQ)
    with TileContext(nc) as tc:
        _trace(nc, tc, stage)
    nc.compile()
    return nc


def _trace(nc, tc, stage='full'):
    with contextlib.ExitStack() as ctx:
        dram = ctx.enter_context(tc.tile_pool(name="dram", bufs=1, space="DRAM"))
        consts = ctx.enter_context(tc.tile_pool(name="consts", bufs=1))
        persist = ctx.enter_context(tc.tile_pool(name="persist", bufs=1))
        sb = ctx.enter_context(tc.tile_pool(name="sb", bufs=2))
        keyp = ctx.enter_context(tc.tile_pool(name="keyp", bufs=2))
        gath = ctx.enter_context(tc.tile_pool(name="gath", bufs=3))
        mp = ctx.enter_context(tc.tile_pool(name="mp", bufs=1))
        psb = ctx.enter_context(tc.tile_pool(name="psb", bufs=3, space="PSUM"))
        pss = ctx.enter_context(tc.tile_pool(name="pss", bufs=2, space="PSUM"))

        # ---- DRAM I/O ----
        x_d = dram.tile([N, 3], F32, kind="ExternalInput", uniquify=False, name="x")
        win = {}
        for li, (C, O) in enumerate(LAYERS, start=1):
            win[f'wnt{li}'] = dram.tile([C, O], F32, kind="ExternalInput", uniquify=False, name=f"wnt{li}")
            win[f'wdt{li}'] = dram.tile([C, O], F32, kind="ExternalInput", uniquify=False, name=f"wdt{li}")
            win[f'bs{li}'] = dram.tile([1, O], F32, kind="ExternalInput", uniquify=False, name=f"bs{li}")
        w5t_d = dram.tile([512, 512], F32, kind="ExternalInput", uniquify=False, name="w5t")
        b5_d = dram.tile([1, 512], F32, kind="ExternalInput", uniquify=False, name="b5")
        wfc1_d = dram.tile([512, 256], F32, kind="ExternalInput", uniquify=False, name="wfc1")
        bfc1_d = dram.tile([128, 2], F32, kind="ExternalInput", uniquify=False, name="bfc1")
        wfc2_d = dram.tile([256, 128], F32, kind="ExternalInput", uniquify=False, name="wfc2")
        bfc2_d = dram.tile([128, 1], F32, kind="ExternalInput", uniquify=False, name="bfc2")
        wfc3_d = dram.tile([128, 40], F32, kind="ExternalInput", uniquify=False, name="wfc3")
        bfc3_d = dram.tile([1, 40], F32, kind="ExternalInput", uniquify=False, name="bfc3")
        out_d = dram.tile([40, 1], F32, kind="ExternalOutput", uniquify=False, name="out")
        dbg_d = None
        if stage != 'full':
            dbg_d = dram.tile([128, N], F32, kind="ExternalOutput", uniquify=False, name="dbg")

        a_ds = {li: dram.tile([N, O], F16 if li == 4 else F32, name=f"a_d{li}")
                for li, (C, O) in enumerate(LAYERS, start=1)}
        jw_ds = {li: dram.tile([N * KNN // 16, 128], I16, name=f"jw_d{li}")
                 for li in range(1, 5)}

        # ---- consts ----
        iotaJ = consts.tile([128, N], U32, tag="iotaJ")
        nc.gpsimd.iota(iotaJ[:, :], [[1, N]], base=0, channel_multiplier=0)
        ident = consts.tile([128, 128], F32, tag="ident")
        make_identity(nc, ident[:, :])
        onescol = consts.tile([128, 1], F32, tag="onescol")
        nc.vector.memset(onescol[:, :], 1.0)
        onesrow = consts.tile([1, N], F32, tag="onesrow")
        nc.vector.memset(onesrow[:, :], 1.0)
        negones = consts.tile([1, N], F32, tag="negones")
        nc.vector.memset(negones[:, :], -1.0)
        maskc = consts.tile([128, 1], U32, tag="maskc")
        nc.vector.memset(maskc[:, :], 0xFFFFFC00)

        # persistent feature tensors
        x0T = persist.tile([3, N], F32, tag="x0T")
        x1T = persist.tile([64, N], F32, tag="x1T")
        x2T = persist.tile([64, N], F32, tag="x2T")
        x3T = persist.tile([128, N], F32, tag="x3T")
        x4Ta = persist.tile([128, N], F32, tag="x4Ta")
        x4Tb = persist.tile([128, N], F32, tag="x4Tb")

        # load x transposed: x_d is (N, 3) row-major
        xap = x_d[:, :]
        nc.sync.dma_start(
            x0T[:, :], bass.AP(xap.tensor, xap.offset, [[1, 3], [3, N]]))
        if stage == 'xload':
            nc.sync.dma_start(dbg_d[0:3, :], x0T[:, :])
            nc.sync.dma_start(out_d[:, :], onescol[0:40, :])
            return

        def edge_layer(li, xT, C, O, out_parts, post_tiles=None):
            a_d = a_ds[li]
            jw_d = jw_ds[li]
            jwap = jw_d[:, :]
            wnt = sb.tile([C, O], F32, tag="wnt")
            wdt = sb.tile([C, O], F32, tag="wdt")
            bs = sb.tile([1, O], F32, tag="bs")
            nc.sync.dma_start(wnt[:, :], win[f'wnt{li}'][:, :])
            nc.sync.dma_start(wdt[:, :], win[f'wdt{li}'][:, :])
            nc.sync.dma_start(bs[:, :], win[f'bs{li}'][:, :])

            jf = sb.tile([128, NT * KNN], F32, tag="jf")
            xsq = sb.tile([C, N], F32, tag="xsq")
            nc.scalar.activation(xsq[:, :], xT[:, :], AF.Square)
            x2d = sb.tile([C, N], F32, tag="x2d")
            nc.scalar.activation(x2d[:, :], xT[:, :], AF.Copy, bias=0.0, scale=2.0)

            SO2 = mp.tile([2, N], F32, tag="SO2")
            OS2 = mp.tile([2, N], F32, tag="OS2")
            negS = mp.tile([1, N], F32, tag="negS")
            nc.vector.memset(OS2[0:1, :], -1.0)
            for h in range(2):
                cols = slice(h * 512, (h + 1) * 512)
                S_ps = pss.tile([1, 512], F32, tag="a")
                nc.tensor.matmul(
                    S_ps[:, :], lhsT=onescol[0:C, :], rhs=xsq[:, cols],
                    start=True, stop=True, skip_group_check=True)
                nc.scalar.activation(
                    SO2[0:1, cols], S_ps[:, :], AF.Copy, bias=0.0, scale=1.0)
                nc.scalar.activation(
                    negS[0:1, cols], S_ps[:, :], AF.Copy, bias=0.0, scale=-1.0)
            nc.sync.dma_start(SO2[1:2, :], onesrow[0:1, :])
            nc.sync.dma_start(OS2[1:2, :], negS[0:1, :])

            # a-rows to DRAM first so gathers can start as soon as idx ready
            for t in range(NT):
                a_ps = pss.tile([128, O], F32, tag="a")
                nc.tensor.matmul(
                    a_ps[:, :], lhsT=xT[:, t * 128:(t + 1) * 128], rhs=wnt[:, :],
                    start=True, stop=True, skip_group_check=True)
                a_sb = sb.tile([128, O], F16 if li == 4 else F32, tag="a_sb")
                nc.scalar.activation(a_sb[:, :], a_ps[:, :], AF.Copy)
                nc.sync.dma_start(a_d[t * 128:(t + 1) * 128, :], a_sb[:, :])

            m = mp.tile([128, NT, O], F32, tag="m")
            pending_g = []
            ntl = NT
            if stage.startswith('topk1_'):
                ntl = int(stage.split('_')[1])
            for t in range(ntl):
                if stage in ('keys', 'pack', 'max1', 'mr1', 'ext', 'topkt0') and t > 0:
                    break
                tcols = slice(t * 128, (t + 1) * 128)
                kp = psb.tile([128, N], F32, tag="big")
                for h in range(2):
                    cols = slice(h * 512, (h + 1) * 512)
                    nc.tensor.matmul(
                        kp[:, cols], lhsT=xT[:, tcols], rhs=x2d[:, cols],
                        start=True, stop=False, skip_group_check=True)
                    nc.tensor.matmul(
                        kp[:, cols], lhsT=SO2[:, tcols], rhs=OS2[:, cols],
                        start=False, stop=True, skip_group_check=True)
                if stage == 'keys' and t == 0:
                    kcp = sb.tile([128, N], F32, tag="kcp")
                    nc.scalar.activation(kcp[:, :], kp[:, :], AF.Copy)
                    nc.sync.dma_start(dbg_d[:, :], kcp[:, :])
                    nc.sync.dma_start(out_d[:, :], onescol[0:40, :])
                    return 'stop'
                kb = keyp.tile([128, N], U32, tag="keysP")
                nc.vector.scalar_tensor_tensor(
                    out=kb[:, :], in0=kp[:, :].bitcast(U32), scalar=maskc[:, 0:1],
                    in1=iotaJ[:, :], op0=ALU.bitwise_and, op1=ALU.bitwise_or)
                if stage == 'pack' and t == 0:
                    kcp = sb.tile([128, N], F32, tag="kcp")
                    nc.vector.tensor_copy(kcp[:, :].bitcast(U32), kb[:, :])
                    nc.sync.dma_start(dbg_d[:, :], kcp[:, :])
                    nc.sync.dma_start(out_d[:, :], onescol[0:40, :])
                    return 'stop'
                kbf = kb[:, :].bitcast(F32)
                v24 = sb.tile([128, 24], F32, tag="v24")
                nc.vector.max(v24[:, 0:8], kbf)
                if stage == 'max1' and t == 0:
                    nc.sync.dma_start(dbg_d[:, 0:8], v24[:, 0:8])
                    nc.sync.dma_start(out_d[:, :], onescol[0:40, :])
                    return 'stop'
                nc.vector.match_replace(kbf, v24[:, 0:8], kbf, NEG_BIG)
                if stage == 'mr1' and t == 0:
                    nc.sync.dma_start(dbg_d[:, :], kb[:, :].bitcast(F32))
                    nc.sync.dma_start(out_d[:, :], onescol[0:40, :])
                    return 'stop'
                nc.vector.max(v24[:, 8:16], kbf)
                nc.vector.match_replace(kbf, v24[:, 8:16], kbf, NEG_BIG)
                nc.vector.max(v24[:, 16:24], kbf)
                if stage == 'ext' and t == 0:
                    nc.sync.dma_start(dbg_d[:, 0:24], v24[:, :])
                    nc.sync.dma_start(out_d[:, :], onescol[0:40, :])
                    return 'stop'
                j20 = sb.tile([128, KNN], U32, tag="j20")
                nc.vector.tensor_scalar(
                    j20[:, :], v24[:, 0:KNN].bitcast(U32), 0x3FF, None,
                    op0=ALU.bitwise_and)
                # j as fp32 values, accumulated for PE transpose
                nc.vector.tensor_copy(jf[:, t * KNN:(t + 1) * KNN], j20[:, :])
                if stage == 'topkt0' and t == 0:
                    nc.sync.dma_start(dbg_d[:, 0:KNN], jf[:, 0:KNN])
                    nc.sync.dma_start(out_d[:, :], onescol[0:40, :])
                    return 'stop'

                # per-tile idx wrap + gather on queue t%NQ
                jT_ps = pss.tile([KNN, 128], F32, tag="a")
                nc.tensor.matmul(
                    jT_ps[:, :], lhsT=jf[:, t * KNN:(t + 1) * KNN],
                    rhs=ident[:, 0:128], is_transpose=True, start=True, stop=True,
                    skip_group_check=True)
                jTi = sb.tile([KNN, 128], I16, tag="jTi")
                nc.vector.tensor_copy(jTi[:, :], jT_ps[:, :])
                dst = bass.AP(jwap.tensor, jwap.offset + t * 160 * 128,
                              [[1024, KNN], [128, 8], [1, 16]])
                nc.sync.dma_start(
                    dst, jTi[:, :].rearrange("k (h s) -> k h s", s=16))
                src_ap = bass.AP(jwap.tensor, jwap.offset + t * 160 * 128,
                                 [[128, 160], [1, 128]])
                idq = keyp.tile([128, 10 * 16], I16, tag="idxq", bufs=4)
                nc.scalar.dma_start_transpose(idq[:, :], src_ap)
                for rr in range(1, 8):
                    nc.scalar.dma_start(
                        idq[16 * rr:16 * (rr + 1), :], idq[0:16, :])

                gdt = F16 if li == 4 else F32
                g = gath.tile([128, KNN, O], gdt, tag="g", bufs=6)
                nc.gpsimd.dma_gather(
                    out_ap=g[:, :, :], in_ap=a_d[:, :],
                    idxs_ap=idq[:, :],
                    num_idxs=KNN * 128, num_idxs_reg=KNN * 128, elem_size=O,
                    single_packet=False, queue_num=t % NQ)
                pending_g.append((t, g))
                # lag the reduce 4 tiles so the DVE stream never head-of-line
                # blocks on an in-flight gather (idx chain + gather ~ 28us)
                if len(pending_g) > 4:
                    tp, gp = pending_g.pop(0)
                    gap = gp[:, :, :]
                    red_in = bass.AP(
                        gap.tensor, gap.offset,
                        [gap.ap[0], [1, O], [O, KNN]])
                    nc.vector.tensor_reduce(
                        out=m[:, tp, :], in_=red_in, axis=AX.X, op=ALU.max)

            for tp, gp in pending_g:
                gap = gp[:, :, :]
                red_in = bass.AP(
                    gap.tensor, gap.offset,
                    [gap.ap[0], [1, O], [O, KNN]])
                nc.vector.tensor_reduce(
                    out=m[:, tp, :], in_=red_in, axis=AX.X, op=ALU.max)

            if stage.startswith('topk1'):
                nc.sync.dma_start(dbg_d[0:128, 0:NT * KNN], jf[:, :])
                nc.sync.dma_start(out_d[:, :], onescol[0:40, :])
                return 'stop'

            if post_tiles is not None:
                post_tiles()

            # transpose m + c matmul + lrelu -> out_parts
            for ot, (op_ap, orow) in enumerate(out_parts):
                px = psb.tile([orow, N], F32, tag="big")
                for t in range(NT):
                    nc.tensor.matmul(
                        px[:, t * 128:(t + 1) * 128],
                        lhsT=m[:, t, ot * 128:ot * 128 + orow],
                        rhs=ident[:, 0:128],
                        is_transpose=True, start=(t % 4 == 0), stop=False,
                        skip_group_check=True)
                for h in range(2):
                    cols = slice(h * 512, (h + 1) * 512)
                    nc.tensor.matmul(
                        px[:, cols],
                        lhsT=wdt[:, ot * 128:ot * 128 + orow],
                        rhs=xT[:, cols],
                        start=False, stop=False, skip_group_check=True)
                    nc.tensor.matmul(
                        px[:, cols],
                        lhsT=bs[0:1, ot * 128:ot * 128 + orow],
                        rhs=onesrow[0:1, cols],
                        start=False, stop=True, skip_group_check=True)
                nc.scalar.activation(op_ap, px[:, :], AF.Prelu, alpha=NEG_SLOPE)

        w5sb = {}
        for ci, (rows, k0) in enumerate([(64, 0), (64, 64), (128, 128),
                                         (128, 256), (128, 384)]):
            w5c = consts.tile([rows, 512], F32, tag=f"w5c{ci}")
            nc.sync.dma_start(w5c[:, :], w5t_d[k0:k0 + rows, :])
            w5sb[ci] = w5c
        b5sb = consts.tile([1, 512], F32, tag="b5sb")
        nc.sync.dma_start(b5sb[:, :], b5_d[:, :])
        zpart = persist.tile([128, NT, 512], F32, tag="zpart")

        def zpart_fill():
            for t in range(NT):
                tcols = slice(t * 128, (t + 1) * 128)
                zp_ps = pss.tile([128, 512], F32, tag="a")
                for ci, (xt, rows) in enumerate(
                        [(x1T, 64), (x2T, 64), (x3T, 128)]):
                    nc.tensor.matmul(
                        zp_ps[:, :], lhsT=xt[:, tcols], rhs=w5sb[ci][:, :],
                        start=(ci == 0), stop=(ci == 2), skip_group_check=True)
                nc.scalar.activation(zpart[:, t, :], zp_ps[:, :], AF.Copy)

        r = edge_layer(1, x0T, 3, 64, [(x1T[:, :], 64)])
        if r == 'stop':
            return
        if stage == 'gath1':
            nc.sync.dma_start(dbg_d[0:64, :], x1T[:, :])
            nc.sync.dma_start(out_d[:, :], onescol[0:40, :])
            return
        edge_layer(2, x1T, 64, 64, [(x2T[:, :], 64)])
        edge_layer(3, x2T, 64, 128, [(x3T[:, :], 128)])
        edge_layer(4, x3T, 128, 256, [(x4Ta[:, :], 128), (x4Tb[:, :], 128)],
                   post_tiles=zpart_fill)

        # ---- head: conv5 (x4 chunks; x1-x3 partials precomputed) + max pool ----
        zmax = persist.tile([128, 512], F32, tag="zmax")
        for t in range(NT):
            tcols = slice(t * 128, (t + 1) * 128)
            z_ps = pss.tile([128, 512], F32, tag="a")
            for ci, (xt, rows, k0) in enumerate(
                    [(x4Ta, 128, 256), (x4Tb, 128, 384)]):
                nc.tensor.matmul(
                    z_ps[:, :], lhsT=xt[:, tcols], rhs=w5sb[3 + ci][:, :],
                    start=(ci == 0), stop=False, skip_group_check=True)
            nc.tensor.matmul(
                z_ps[:, :], lhsT=onesrow[0:1, tcols],
                rhs=b5sb[:, :], start=False, stop=True, skip_group_check=True)
            zsb = sb.tile([128, 512], F32, tag="zsb")
            nc.vector.tensor_tensor(
                out=zsb[:, :], in0=zpart[:, t, :], in1=z_ps[:, :], op=ALU.add)
            if t == 0:
                nc.scalar.activation(zmax[:, :], zsb[:, :], AF.Copy)
            else:
                nc.vector.tensor_tensor(
                    out=zmax[:, :], in0=zmax[:, :], in1=zsb[:, :], op=ALU.max)
        # partition tree-max 128 -> 1... then we need yT [128, 4] instead:
        # transpose zmax chunks and reduce along free dim.
        yT = persist.tile([128, 4], F32, tag="yT")
        for cchunk in range(4):
            zt_ps = pss.tile([128, 128], F32, tag="a")
            nc.tensor.matmul(
                zt_ps[:, :], lhsT=zmax[:, cchunk * 128:(cchunk + 1) * 128],
                rhs=ident[:, 0:128], is_transpose=True, start=True, stop=True,
                skip_group_check=True)
            nc.vector.tensor_reduce(
                out=yT[:, cchunk:cchunk + 1], in_=zt_ps[:, :],
                axis=AX.X, op=ALU.max)
        # leaky relu on yT
        yTr = persist.tile([128, 4], F32, tag="yTr")
        nc.scalar.activation(yTr[:, :], yT[:, :], AF.Prelu, alpha=NEG_SLOPE)

        # ---- FC head ----
        wfc1sb = consts.tile([128, 4, 256], F32, tag="wfc1sb")
        for c in range(4):
            nc.sync.dma_start(wfc1sb[:, c, :], wfc1_d[c * 128:(c + 1) * 128, :])
        bfc1sb = consts.tile([128, 2], F32, tag="bfc1sb")
        nc.sync.dma_start(bfc1sb[:, :], bfc1_d[:, :])
        wfc2sb = consts.tile([128, 2, 128], F32, tag="wfc2sb")
        for c in range(2):
            nc.sync.dma_start(wfc2sb[:, c, :], wfc2_d[c * 128:(c + 1) * 128, :])
        bfc2sb = consts.tile([128, 1], F32, tag="bfc2sb")
        nc.sync.dma_start(bfc2sb[:, :], bfc2_d[:, :])
        wfc3sb = consts.tile([128, 40], F32, tag="wfc3sb")
        nc.sync.dma_start(wfc3sb[:, :], wfc3_d[:, :])
        bfc3sb = consts.tile([1, 40], F32, tag="bfc3sb")
        nc.sync.dma_start(bfc3sb[:, :], bfc3_d[:, :])

        h1sb = persist.tile([128, 2], F32, tag="h1sb")
        for mt in range(2):
            h1_ps = pss.tile([128, 1], F32, tag="a")
            for c in range(4):
                nc.tensor.matmul(
                    h1_ps[:, :], lhsT=wfc1sb[:, c, mt * 128:(mt + 1) * 128],
                    rhs=yTr[:, c:c + 1],
                    start=(c == 0), stop=(c == 3), skip_group_check=True)
            nc.scalar.activation(
                h1sb[:, mt:mt + 1], h1_ps[:, :], AF.Prelu,
                bias=bfc1sb[:, mt:mt + 1], scale=1.0, alpha=NEG_SLOPE)
        h2sb = persist.tile([128, 1], F32, tag="h2sb")
        h2_ps = pss.tile([128, 1], F32, tag="a")
        for c in range(2):
            nc.tensor.matmul(
                h2_ps[:, :], lhsT=wfc2sb[:, c, :], rhs=h1sb[:, c:c + 1],
                start=(c == 0), stop=(c == 1), skip_group_check=True)
        nc.scalar.activation(
            h2sb[:, :], h2_ps[:, :], AF.Prelu,
            bias=bfc2sb[:, :], scale=1.0, alpha=NEG_SLOPE)

        out_ps = pss.tile([40, 1], F32, tag="a")
        nc.tensor.matmul(
            out_ps[:, :], lhsT=wfc3sb[:, :], rhs=h2sb[:, :],
            start=True, stop=False, skip_group_check=True)
        nc.tensor.matmul(
            out_ps[:, :], lhsT=bfc3sb[:, :], rhs=onescol[0:1, :],
            start=False, stop=True, skip_group_check=True)
        out_sb = persist.tile([40, 1], F32, tag="out_sb")
        nc.scalar.activation(out_sb[:, :], out_ps[:, :], AF.Copy)
        nc.sync.dma_start(out_d[:, :], out_sb[:, :])


# ---------------------------------------------------------------------------
# harness entry point
# ---------------------------------------------------------------------------
_NC_CACHE = {}


def _get_nc():
    if 'nc' not in _NC_CACHE:
        _NC_CACHE['nc'] = build_nc()
    return _NC_CACHE['nc']


def kernel(**inputs):
    """Full-batch EdgeCNN forward. x: (8, 1024, 3) -> (8, 40) float32.

    Pure data parallel: batch element b runs on NeuronCore b.
    """
    from concourse.bass_utils import run_bass_kernel_spmd

    inp = {k: np.asarray(v) for k, v in inputs.items()}
    prep = host_prep(inp)
    nc = _get_nc()
    in_maps = []
    for b in range(8):
        m = {'x': np.ascontiguousarray(inp['x'][b]).astype(np.float32)}
        m.update(prep)
        in_maps.append(m)
    res = run_bass_kernel_spmd(nc, in_maps, core_ids=list(range(8)))
    out = np.stack([res.results[b]['out'].reshape(40) for b in range(8)])
    return out.astype(np.float32)
